# revision 1
# baseline (speedup 1.0000x reference)
"""Trainium2 Bass kernel for nn_BasicDeconvolutionBlock.

Reference computation (see problem statement):
    gathered = feats[in_map]                         # [K, M, Cin]
    contrib  = einsum('kmc,kcd->kmd', gathered, W)   # [K, M, Cout]
    out      = zeros([n_out, Cout]).at[out_map].add(contrib)
    y        = relu(batchnorm(out))                  # batch stats over n_out rows

Strategy (8 NeuronCores, SPMD):
  - Host routes each (k, m) pair to the core owning its output row
    (row blocks of n_out/8).  Per core ~169k pairs.
  - Gather: feats pre-cast to bf16, padded to 128 channels (256B rows).
    dma_gather(transpose=True) produces a CHANNEL-MAJOR SBUF slab
    G[128ch, slots] directly.  int16 gather indices -> feats is split in
    chunks of 32768 rows; pairs are grouped by (chunk, k), groups padded
    to a multiple of 128 slots.
  - GEMM: per 128-slot tile, matmul(lhsT=G_tile[128ch,128slots] (stationary),
    rhs=Wpad[k][128ch,64]) -> PSUM contrib[128slots, 64] fp32 (m-major,
    no transposes anywhere).
  - Scatter: DVE copies PSUM->SBUF slab, then gpsimd dma_scatter_add
    (CCE-add, int16 idx) accumulates rows into one of two HBM accumulator
    banks (cycled by round parity so chains overlap).  Duplicate rows race
    in hardware, so a host-side occurrence-round split guarantees unique
    rows per call; same-bank calls serialize via Tile WAW deps.  SWDGE
    calls are capped at 896 indices (the Q7 ucode descriptor-ring limit;
    larger calls hard-wedge the device).
  - BN: ones-matmul row sums + sum of squares, [2,64] AllReduce across
    the 8 cores, normalize + ReLU on chip, output shard [rows,64] fp32.
"""

import os
import sys

import numpy as np

sys.path.insert(0, "/opt/trn_rl_repo")

import ml_dtypes  # noqa: E402

from concourse import bacc, bass, mybir  # noqa: E402
import concourse.tile as tile  # noqa: E402

BN_EPS = 1e-5
CHUNK = 32768  # int16 gather index range per feats chunk
SEG_SLOTS = 896  # max slots per SWDGE call; 1024+ wedges the device (Q7 ucode descriptor-ring limit, verified empirically)
F32 = mybir.dt.float32
BF16 = mybir.dt.bfloat16
I16 = mybir.dt.int16
I32 = mybir.dt.int32


def _roundup(x, m):
    return (x + m - 1) // m * m


def _route(in_map, out_map, n_out, n_cores, dup_safe, expand=1):
    """Host-side routing. Returns compile-time plan + per-core packed arrays.

    Slot stream per core: for r in rounds, for c in chunks, for k in K:
    group (r,c,k) padded to a multiple of 128 slots.  If dup_safe, a single
    round (r=0) is used (occurrence splitting disabled).

    expand=E spreads a row's duplicate contributions over E contiguous
    accumulator banks (phys row = (occ%E)*acc_rows + row, round = occ//E),
    halving/quartering the round count; the kernel folds banks before BN.
    """
    K, M = in_map.shape
    rows_per_core = n_out // n_cores
    assert rows_per_core * n_cores == n_out
    acc_rows = _roundup(rows_per_core, 128)
    nchunk = _roundup(int(in_map.max()) + 1, CHUNK) // CHUNK

    k_idx = np.repeat(np.arange(K, dtype=np.int32), M)
    in_flat = in_map.ravel().astype(np.int64)
    out_flat = out_map.ravel().astype(np.int64)
    core = out_flat // rows_per_core
    row_local = (out_flat - core * rows_per_core).astype(np.int32)
    chunk = (in_flat // CHUNK).astype(np.int32)
    idx_local = (in_flat - chunk.astype(np.int64) * CHUNK).astype(np.int32)

    per_core = []
    max_round = 1
    for c in range(n_cores):
        sel = np.nonzero(core == c)[0]
        rows_c = row_local[sel]
        if dup_safe:
            rnd = np.zeros(len(sel), dtype=np.int32)
            prow = rows_c.astype(np.int32)
        else:
            order = np.argsort(rows_c, kind="stable")
            sr = rows_c[order]
            n = len(sr)
            first = np.ones(n, dtype=bool)
            first[1:] = sr[1:] != sr[:-1]
            grp_start = np.maximum.accumulate(np.where(first, np.arange(n), 0))
            occ_sorted = np.arange(n) - grp_start
            occ = np.empty(n, dtype=np.int64)
            occ[order] = occ_sorted
            rnd = (occ // expand).astype(np.int32)
            prow = (rows_c + (occ % expand) * acc_rows).astype(np.int32)
            max_round = max(max_round, int(rnd.max()) + 1 if n else 1)
        per_core.append(
            dict(rnd=rnd, chunk=chunk[sel], k=k_idx[sel],
                 idx=idx_local[sel], row=prow)
        )

    R = max_round
    # group counts [R, nchunk, K] per core -> shared caps
    counts = np.zeros((n_cores, R, nchunk, K), dtype=np.int64)
    for c in range(n_cores):
        p = per_core[c]
        np.add.at(counts[c], (p["rnd"], p["chunk"], p["k"]), 1)
    caps = (np.ceil(counts.max(axis=0) / 128).astype(np.int64) * 128)  # [R,nchunk,K]

    # segments: contiguous runs of (r,c,k) group pieces, same (r,c),
    # <= SEG_SLOTS per segment (SWDGE per-instruction descriptor limit).
    # Groups larger than SEG_SLOTS are split across segments.
    segments = []  # dicts: r, c, slot0 (global), nslots, groups=[(k, len, off_in_seg)]
    group_slot0 = {}  # (r,c,k) -> global slot of the group's first slot
    slot0 = 0
    for r in range(R):
        for c in range(nchunk):
            cur = None
            for k in range(K):
                cap = int(caps[r, c, k])
                if cap == 0:
                    continue
                group_slot0[(r, c, k)] = slot0 + (cur["nslots"] if cur else 0)
                rem = cap
                while rem > 0:
                    if cur is None:
                        cur = dict(r=r, c=c, slot0=slot0, nslots=0, groups=[])
                    take = min(SEG_SLOTS - cur["nslots"], rem)
                    if take == 0:
                        segments.append(cur)
                        slot0 += cur["nslots"]
                        cur = None
                        continue
                    cur["groups"].append((k, take, cur["nslots"]))
                    cur["nslots"] += take
                    rem -= take
            if cur is not None:
                segments.append(cur)
                slot0 += cur["nslots"]
                cur = None
    total_slots = slot0

    dump_row = expand * acc_rows  # rows beyond the banks are the dump zone
    acc_total = expand * acc_rows + 128

    # pack per-core gather idx and scatter idx (both int16, wrapped 16)
    gcols = sum(seg["nslots"] // 16 for seg in segments)
    scols = gcols
    gidx_all = np.zeros((n_cores, 128, gcols), dtype=np.int16)
    sidx_all = np.full((n_cores, 128, scols), dump_row, dtype=np.int16)

    seg_gcol0 = []
    seg_scol0 = []
    g0 = s0 = 0
    for seg in segments:
        seg_gcol0.append(g0)
        seg_scol0.append(s0)
        g0 += seg["nslots"] // 16
        s0 += seg["nslots"] // 16

    for cidx in range(n_cores):
        p = per_core[cidx]
        order = np.lexsort((p["row"], p["k"], p["chunk"], p["rnd"]))
        rnd_s, ch_s, k_s = p["rnd"][order], p["chunk"][order], p["k"][order]
        idx_s, row_s = p["idx"][order], p["row"][order]
        # slot of each pair: group_slot0 + position within group
        key = (rnd_s.astype(np.int64) * nchunk + ch_s) * K + k_s
        n = len(key)
        first = np.ones(n, dtype=bool)
        first[1:] = key[1:] != key[:-1]
        grp_start = np.maximum.accumulate(np.where(first, np.arange(n), 0))
        pos_in_grp = np.arange(n) - grp_start
        base = np.array(
            [group_slot0[(int(r_), int(c_), int(k_))]
             for r_, c_, k_ in zip(rnd_s[first], ch_s[first], k_s[first])],
            dtype=np.int64,
        )
        base_full = np.repeat(base, np.diff(np.nonzero(
            np.concatenate([first, [True]]))[0]))
        slots = base_full + pos_in_grp

        gvals = np.zeros(total_slots, dtype=np.int16)
        svals = np.full(total_slots, dump_row, dtype=np.int16)
        gvals[slots] = idx_s.astype(np.int16)
        svals[slots] = row_s
        # per-segment packing
        for si, seg in enumerate(segments):
            a, b = seg["slot0"], seg["slot0"] + seg["nslots"]
            gseg = gvals[a:b].reshape(-1, 16).T  # [16, n/16]
            gidx_all[cidx, :, seg_gcol0[si]:seg_gcol0[si] + (b - a) // 16] = (
                np.tile(gseg, (8, 1)))
            sseg = np.tile(svals[a:b].astype(np.int16).reshape(-1, 16).T,
                           (8, 1))  # wrapped like gather idxs
            sidx_all[cidx, :, seg_scol0[si]:seg_scol0[si] + (b - a) // 16] = sseg

    plan = dict(
        R=R, nchunk=nchunk, K=K, rows_per_core=rows_per_core,
        acc_rows=acc_rows, acc_total=acc_total, dump_row=dump_row,
        expand=expand,
        segments=segments, seg_gcol0=seg_gcol0, seg_scol0=seg_scol0,
        gcols=gcols, scols=scols, total_slots=total_slots,
    )
    return plan, gidx_all, sidx_all


def _build(plan, n_out, ftab_rows, n_cores):
    """Trace the Bass program. Returns nc."""
    nc = bacc.Bacc("TRN2", target_bir_lowering=False, debug=False)

    R, nchunk, K = plan["R"], plan["nchunk"], plan["K"]
    acc_rows, acc_total = plan["acc_rows"], plan["acc_total"]
    segments = plan["segments"]
    Cout = 64

    ftab = nc.dram_tensor("ftab", [ftab_rows, 128], BF16, kind="ExternalInput")
    wt = nc.dram_tensor("wt", [128, K * Cout], BF16, kind="ExternalInput")
    gidx = nc.dram_tensor("gidx", [128, plan["gcols"]], I16, kind="ExternalInput")
    sidx = nc.dram_tensor("sidx", [128, plan["scols"]], I16, kind="ExternalInput")
    gb = nc.dram_tensor("gb", [2, Cout], F32, kind="ExternalInput")
    # two accumulator banks cycled by round parity: scatter calls to
    # different banks have no WAW conflict, so adjacent rounds overlap
    acc0 = nc.dram_tensor("acc0", [acc_total, Cout], F32)
    acc1 = nc.dram_tensor("acc1", [acc_total, Cout], F32)
    accs = [acc0, acc1]
    cc_in = nc.dram_tensor("cc_in", [2, Cout], F32)
    cc_out = nc.dram_tensor("cc_out", [2, Cout], F32, addr_space="Shared")
    y = nc.dram_tensor("y", [acc_rows, Cout], F32, kind="ExternalOutput")

    Tb = acc_rows // 128  # BN column tiles

    with tile.TileContext(nc) as tc:
        with (
            tc.tile_pool(name="const", bufs=1) as cpool,
            tc.tile_pool(name="gpool", bufs=3) as gpool,
            tc.tile_pool(name="slab", bufs=3) as slabpool,
            tc.tile_pool(name="gixp", bufs=3) as gixpool,
            tc.tile_pool(name="sixp", bufs=3) as sixpool,
            tc.tile_pool(name="psum", bufs=8, space="PSUM") as pspool,
        ):
            # constants
            w_sb = cpool.tile([128, K * Cout], BF16, tag="w")
            nc.sync.dma_start(out=w_sb[:, :], in_=wt[:, :])
            zed = cpool.tile([128, 3200], F32, tag="zed")
            nc.vector.memset(zed[:, :], 0.0)
            # zero-init acc (acc_total*64 elems, in chunks of 128*3200)
            zrows = 128 * 3200 // Cout  # 6400 rows per DMA
            for bank in accs:
                r0 = 0
                while r0 < acc_total:
                    rcnt = min(zrows, acc_total - r0)
                    nc.sync.dma_start(
                        out=bank[r0:r0 + rcnt, :],
                        in_=zed[:, :rcnt * Cout // 128],
                    )
                    r0 += rcnt

            # main pipeline over segments
            for si, seg in enumerate(segments):
                ns = seg["nslots"]
                c = seg["c"]
                gi = gixpool.tile([128, SEG_SLOTS // 16], I16, tag="gi")
                nc.sync.dma_start(
                    out=gi[:, :ns // 16],
                    in_=gidx[:, plan["seg_gcol0"][si]:plan["seg_gcol0"][si] + ns // 16],
                )
                g = gpool.tile([128, 1, SEG_SLOTS], BF16, tag="g")
                nc.gpsimd.dma_gather(
                    out_ap=g[:, :, :ns],
                    in_ap=ftab[c * CHUNK:min((c + 1) * CHUNK, ftab_rows), :],
                    idxs_ap=gi[:, :ns // 16],
                    num_idxs=ns,
                    num_idxs_reg=ns,
                    elem_size=128,
                    transpose=True,
                )
                slab = slabpool.tile([128, SEG_SLOTS // 128, Cout], F32, tag="slab")
                for (k, cap, off) in seg["groups"]:
                    for j in range(cap // 128):
                        col = off + j * 128
                        ps = pspool.tile([128, Cout], F32, tag="ps")
                        nc.tensor.matmul(
                            out=ps[:, :],
                            lhsT=g[:, 0, col:col + 128],
                            rhs=w_sb[:, k * Cout:(k + 1) * Cout],
                            start=True, stop=True,
                        )
                        nc.vector.tensor_copy(
                            out=slab[:, col // 128, :], in_=ps[:, :])
                si_t = sixpool.tile([128, SEG_SLOTS // 16], I16, tag="si")
                nc.sync.dma_start(
                    out=si_t[:, :ns // 16],
                    in_=sidx[:, plan["seg_scol0"][si]:plan["seg_scol0"][si] + ns // 16],
                )
                nc.gpsimd.dma_scatter_add(
                    out_ap=accs[seg["r"] % 2][:, :],
                    in_ap=slab[:, :ns // 128, :],
                    idxs_ap=si_t[:, :ns // 16],
                    num_idxs=ns,
                    num_idxs_reg=ns,
                    elem_size=64,
                )

        # ---- BN phase ----
        with (
            tc.tile_pool(name="bn", bufs=1) as bnpool,
            tc.tile_pool(name="bns", bufs=4) as bnspool,
            tc.tile_pool(name="bnp", bufs=2, space="PSUM") as bnps,
        ):
            out_sb = bnpool.tile([128, Tb, 64], F32, tag="outsb")
            nc.sync.dma_start(out=out_sb[:, :, :], in_=acc0[0:acc_rows, :])
            bank_sb = bnpool.tile([128, Tb, 64], F32, tag="bank")
            nc.sync.dma_start(out=bank_sb[:, :, :], in_=acc1[0:acc_rows, :])
            nc.vector.tensor_tensor(
                out=out_sb[:, :, :], in0=out_sb[:, :, :],
                in1=bank_sb[:, :, :], op=mybir.AluOpType.add)
            ones = bnpool.tile([128, 1], F32, tag="ones")
            nc.vector.memset(ones[:, :], 1.0)
            sum_ps = bnps.tile([1, 64], F32, tag="sum")
            sq_ps = bnps.tile([1, 64], F32, tag="sq")
            for t in range(Tb):
                nc.tensor.matmul(
                    out=sum_ps[:, :], lhsT=ones[:, :], rhs=out_sb[:, t, :],
                    start=(t == 0), stop=(t == Tb - 1),
                )
            sqt = bnspool.tile([128, 64], F32, tag="sqt")
            for t in range(Tb):
                nc.vector.tensor_tensor(
                    out=sqt[:, :], in0=out_sb[:, t, :], in1=out_sb[:, t, :],
                    op=mybir.AluOpType.mult)
                nc.tensor.matmul(
                    out=sq_ps[:, :], lhsT=ones[:, :], rhs=sqt[:, :],
                    start=(t == 0), stop=(t == Tb - 1),
                )
            st0 = bnspool.tile([1, 64], F32, tag="st0")
            st1 = bnspool.tile([1, 64], F32, tag="st1")
            nc.vector.tensor_copy(out=st0[:, :], in_=sum_ps[:, :])
            nc.vector.tensor_copy(out=st1[:, :], in_=sq_ps[:, :])
            nc.sync.dma_start(out=cc_in[0:1, :], in_=st0[:, :])
            nc.sync.dma_start(out=cc_in[1:2, :], in_=st1[:, :])
            nc.gpsimd.collective_compute(
                "AllReduce",
                mybir.AluOpType.add,
                ins=[cc_in[:, :]],
                outs=[cc_out[:, :]],
                replica_groups=[list(range(n_cores))],
            )
            gs0 = bnspool.tile([1, 64], F32, tag="gs0")
            gs1 = bnspool.tile([1, 64], F32, tag="gs1")
            nc.sync.dma_start(out=gs0[:, :], in_=cc_out[0:1, :])
            nc.sync.dma_start(out=gs1[:, :], in_=cc_out[1:2, :])
            gam_t = bnspool.tile([1, 64], F32, tag="gam")
            bet_t = bnspool.tile([1, 64], F32, tag="bet")
            nc.sync.dma_start(out=gam_t[:, :], in_=gb[0:1, :])
            nc.sync.dma_start(out=bet_t[:, :], in_=gb[1:2, :])

            inv_n = 1.0 / float(n_out)
            mean_t = bnspool.tile([1, 64], F32, tag="mean")
            ex2_t = bnspool.tile([1, 64], F32, tag="ex2")
            var_t = bnspool.tile([1, 64], F32, tag="var")
            sd_t = bnspool.tile([1, 64], F32, tag="sd")
            rs_t = bnspool.tile([1, 64], F32, tag="rs")
            a_t = bnspool.tile([1, 64], F32, tag="a")
            b_t = bnspool.tile([1, 64], F32, tag="b")
            nc.vector.tensor_scalar_mul(mean_t[:, :], gs0[:, :], inv_n)
            nc.vector.tensor_scalar_mul(ex2_t[:, :], gs1[:, :], inv_n)
            nc.vector.tensor_tensor(
                out=var_t[:, :], in0=mean_t[:, :], in1=mean_t[:, :],
                op=mybir.AluOpType.mult)
            nc.vector.tensor_tensor(
                out=var_t[:, :], in0=ex2_t[:, :], in1=var_t[:, :],
                op=mybir.AluOpType.subtract)
            nc.vector.tensor_scalar_add(var_t[:, :], var_t[:, :], BN_EPS)
            nc.scalar.activation(
                out=sd_t[:, :], in_=var_t[:, :],
                func=mybir.ActivationFunctionType.Sqrt)
            nc.vector.reciprocal(out=rs_t[:, :], in_=sd_t[:, :])
            nc.vector.tensor_tensor(
                out=a_t[:, :], in0=gam_t[:, :], in1=rs_t[:, :],
                op=mybir.AluOpType.mult)
            nc.vector.tensor_tensor(
                out=b_t[:, :], in0=mean_t[:, :], in1=a_t[:, :],
                op=mybir.AluOpType.mult)
            nc.vector.tensor_tensor(
                out=b_t[:, :], in0=bet_t[:, :], in1=b_t[:, :],
                op=mybir.AluOpType.subtract)
            # broadcast [1,64] -> [128,64] via PE (ones[1,128]^T @ row)
            ones_row = bnspool.tile([1, 128], F32, tag="ones_row")
            nc.vector.memset(ones_row[:, :], 1.0)
            a_full = bnspool.tile([128, 64], F32, tag="afull")
            b_full = bnspool.tile([128, 64], F32, tag="bfull")
            ab_ps = bnps.tile([128, 64], F32, tag="abps")
            nc.tensor.matmul(
                out=ab_ps[:, :], lhsT=ones_row[:, :], rhs=a_t[:, :],
                start=True, stop=True)
            nc.vector.tensor_copy(out=a_full[:, :], in_=ab_ps[:, :])
            nc.tensor.matmul(
                out=ab_ps[:, :], lhsT=ones_row[:, :], rhs=b_t[:, :],
                start=True, stop=True)
            nc.vector.tensor_copy(out=b_full[:, :], in_=ab_ps[:, :])
            for t in range(Tb):
                nc.vector.tensor_tensor(
                    out=out_sb[:, t, :], in0=out_sb[:, t, :], in1=a_full[:, :],
                    op=mybir.AluOpType.mult)
                nc.vector.tensor_tensor(
                    out=out_sb[:, t, :], in0=out_sb[:, t, :], in1=b_full[:, :],
                    op=mybir.AluOpType.add)
                nc.scalar.activation(
                    out=out_sb[:, t, :], in_=out_sb[:, t, :],
                    func=mybir.ActivationFunctionType.Relu)
            nc.sync.dma_start(out=y[:, :], in_=out_sb[:, :, :])

    nc.compile()
    return nc


def _prepare(feats, W, gamma, beta, in_map, out_map, n_out, n_cores, dup_safe,
             expand=1):
    """Host prep shared by kernel() and tests. Returns (nc, in_maps, plan)."""
    n_out = int(n_out)
    K, Cin, Cout = W.shape
    assert Cin == 64 and Cout == 64
    in_map = np.asarray(in_map, dtype=np.int64)
    out_map = np.asarray(out_map, dtype=np.int64)
    feats = np.asarray(feats, dtype=np.float32)
    W = np.asarray(W, dtype=np.float32)

    plan, gidx_all, sidx_all = _route(
        in_map, out_map, n_out, n_cores, dup_safe, expand)

    ftab_rows = _roundup(feats.shape[0], CHUNK)
    ftab = np.zeros((ftab_rows, 128), dtype=ml_dtypes.bfloat16)
    ftab[:feats.shape[0], :64] = feats.astype(ml_dtypes.bfloat16)

    # W padded: [128 ic, K*64] bf16, rows 64..127 zero
    wt = np.zeros((128, K * 64), dtype=ml_dtypes.bfloat16)
    wt[:64, :] = (
        W.transpose(1, 0, 2).reshape(64, K * 64).astype(ml_dtypes.bfloat16))

    gb = np.stack([np.asarray(gamma, np.float32),
                   np.asarray(beta, np.float32)])

    nc = _build(plan, n_out, ftab_rows, n_cores)
    in_maps = [
        dict(ftab=ftab, wt=wt, gidx=gidx_all[c], sidx=sidx_all[c], gb=gb)
        for c in range(n_cores)
    ]
    return nc, in_maps, plan


def kernel(feats, W, gamma, beta, in_map, out_map, n_out):
    from concourse.bass_utils import run_bass_kernel_spmd

    n_cores = 8
    dup_safe = os.environ.get("DECONV_DUP_SAFE", "0") == "1"
    expand = int(os.environ.get("DECONV_EXPAND", "1"))
    nc, in_maps, plan = _prepare(
        feats, W, gamma, beta, in_map, out_map, n_out, n_cores, dup_safe,
        expand)
    res = run_bass_kernel_spmd(nc, in_maps, list(range(n_cores)))
    rows = plan["rows_per_core"]
    out = np.concatenate(
        [res.results[c]["y"][:rows] for c in range(n_cores)], axis=0)
    return out.astype(np.float32)



# revision 5
# speedup vs baseline: 2.5376x; 2.5376x over previous
"""Trainium2 Bass kernel for nn_BasicDeconvolutionBlock.

Reference computation:
    gathered = feats[in_map]                         # [K, M, Cin]
    contrib  = einsum('kmc,kcd->kmd', gathered, W)   # [K, M, Cout]
    out      = zeros([n_out, Cout]).at[out_map].add(contrib)
    y        = relu(batchnorm(out))                  # batch stats over n_out rows

Strategy (8 NeuronCores, SPMD):
  - Host routes each (k, m) pair to the core owning its output row
    (row blocks of n_out/8) and lowers the gather to im2col: a per-core
    channel-major slab slabT[64, slots] (bf16) holding feats rows in
    k-major slot order, streamed to SBUF with large contiguous DMAs.
  - GEMM: per 128-slot tile (single k per tile), matmul(lhsT=slab tile
    [64ch,128slots], rhs=W[k][64ch,64]) -> PSUM [128slots,64] f32;
    PSUM->SBUF copies alternate between DVE and Activation engines.
  - Scatter: gpsimd dma_scatter_add (CCE-add, int16 idx) accumulates
    contributions into one of two HBM banks, alternating PER SEGMENT so
    adjacent calls have no WAW hazard and fully overlap (desc-gen of
    call i+1 runs during the DMA of call i).  Duplicate output rows
    inside one call race in hardware, so the host spaces a row's
    occurrences ~cnt/h apart in the slot order and swap-fixes the few
    residual in-segment duplicates.  Calls are capped at 896 indices
    (SWDGE Q7 descriptor-ring limit; larger calls wedge the device).
  - BN: fold banks, ones-matmul row sums + sum of squares, [2,64]
    AllReduce across the 8 cores, normalize + ReLU on chip, output
    shard [rows,64] fp32.
"""

import os
import sys

import numpy as np

sys.path.insert(0, "/opt/trn_rl_repo")

import ml_dtypes  # noqa: E402

from concourse import bacc, bass, mybir  # noqa: E402
import concourse.tile as tile  # noqa: E402

BN_EPS = 1e-5
SEG_TILES = 7       # 896 slots per scatter call (SWDGE ring limit)
SUPER_SEGS = 4      # segments per slab/sidx load
F32 = mybir.dt.float32
BF16 = mybir.dt.bfloat16
I16 = mybir.dt.int16


def _roundup(x, m):
    return (x + m - 1) // m * m


def _order_group(rows):
    """Slot order for one (core, k) group: spread a row's occurrences
    ~cnt/h apart so same-call duplicates are rare. Returns a permutation
    of range(len(rows))."""
    n = len(rows)
    if n == 0:
        return np.empty(0, dtype=np.int64)
    order = np.argsort(rows, kind="stable")
    sr = rows[order]
    first = np.ones(n, dtype=bool)
    first[1:] = sr[1:] != sr[:-1]
    grp = np.cumsum(first) - 1                    # rank of unique row
    grp_start = np.maximum.accumulate(np.where(first, np.arange(n), 0))
    occ = np.arange(n) - grp_start                # occurrence index j
    # occurrence count h per element
    cnt_per_grp = np.bincount(grp)
    h = cnt_per_grp[grp]
    nuniq = cnt_per_grp.size
    key = (occ + grp / max(nuniq, 1)) / h
    final = np.argsort(key, kind="stable")
    return order[final]


def _fix_conflicts(svals, gvals, seg_bounds, group_bounds, dump_row):
    """Ensure no duplicate (non-dump) rows within any segment by swapping
    slots within their k-group. svals/gvals modified in place."""
    nslots = len(svals)
    nseg = len(seg_bounds) - 1
    seg_of = np.zeros(nslots, dtype=np.int64)
    for s in range(nseg):
        seg_of[seg_bounds[s]:seg_bounds[s + 1]] = s
    grp_of = np.zeros(nslots, dtype=np.int64)
    for g in range(len(group_bounds) - 1):
        grp_of[group_bounds[g]:group_bounds[g + 1]] = g

    # per-seg row sets
    seg_sets = [set() for _ in range(nseg)]
    conflicts = []
    is_conflict = np.zeros(nslots, dtype=bool)
    for i in range(nslots):
        r = svals[i]
        if r == dump_row:
            continue
        ss = seg_sets[seg_of[i]]
        if r in ss:
            conflicts.append(i)
            is_conflict[i] = True
        else:
            ss.add(r)
    rng = np.random.default_rng(0)
    for i in conflicts:
        r = int(svals[i])
        g = grp_of[i]
        lo, hi = group_bounds[g], group_bounds[g + 1]
        placed = False
        for _ in range(200):
            j = int(rng.integers(lo, hi))
            sj = seg_of[j]
            if sj == seg_of[i] or is_conflict[j]:
                continue
            rj = int(svals[j])
            # after swap: r goes to seg sj, rj comes to seg of i
            if r in seg_sets[sj]:
                continue
            if rj != dump_row:
                if rj == r or rj in seg_sets[seg_of[i]]:
                    continue
            # apply swap
            si = seg_of[i]
            if rj != dump_row:
                seg_sets[sj].discard(rj)
                seg_sets[si].add(rj)
            seg_sets[sj].add(r)
            svals[i], svals[j] = svals[j], svals[i]
            gvals[i], gvals[j] = gvals[j], gvals[i]
            is_conflict[i] = False
            placed = True
            break
        if not placed:
            raise RuntimeError("conflict fix failed")
    return len(conflicts)


def _route(in_map, out_map, n_out, n_cores):
    """Host-side routing. Returns plan + per-core slot arrays
    (gvals: feats row per slot or -1; svals: local out row per slot)."""
    K, M = in_map.shape
    rows_per_core = n_out // n_cores
    assert rows_per_core * n_cores == n_out
    acc_rows = _roundup(rows_per_core, 128)
    dump_row = acc_rows
    acc_total = acc_rows + 128

    in_flat = in_map.ravel().astype(np.int64)
    out_flat = out_map.ravel().astype(np.int64)
    k_idx = np.repeat(np.arange(K, dtype=np.int64), M)
    core = out_flat // rows_per_core
    row_local = (out_flat - core * rows_per_core).astype(np.int64)

    # per (core, k) counts -> shared caps
    counts = np.zeros((n_cores, K), dtype=np.int64)
    np.add.at(counts, (core, k_idx), 1)
    caps = _roundup(counts.max(axis=0), 128)  # [K]
    group_bounds = np.concatenate([[0], np.cumsum(caps)])
    total_slots = int(group_bounds[-1])

    seg_slots = SEG_TILES * 128
    seg_bounds = list(range(0, total_slots, seg_slots)) + [total_slots]
    if seg_bounds[-1] == seg_bounds[-2]:
        seg_bounds.pop()

    per_core = []
    for c in range(n_cores):
        gvals = np.full(total_slots, -1, dtype=np.int64)
        svals = np.full(total_slots, dump_row, dtype=np.int64)
        sel_c = core == c
        for k in range(K):
            sel = np.nonzero(sel_c & (k_idx == k))[0]
            rows_k = row_local[sel]
            perm = _order_group(rows_k)
            g0 = group_bounds[k]
            n = len(sel)
            gvals[g0:g0 + n] = in_flat[sel][perm]
            svals[g0:g0 + n] = rows_k[perm]
        nfix = _fix_conflicts(svals, gvals, seg_bounds, group_bounds,
                              dump_row)
        per_core.append((gvals, svals, nfix))

    # tile -> k map
    ntiles = total_slots // 128
    tile_k = np.zeros(ntiles, dtype=np.int64)
    for k in range(K):
        tile_k[group_bounds[k] // 128:group_bounds[k + 1] // 128] = k

    plan = dict(
        K=K, rows_per_core=rows_per_core, acc_rows=acc_rows,
        acc_total=acc_total, dump_row=dump_row,
        total_slots=total_slots, ntiles=ntiles, tile_k=tile_k,
        seg_bounds=seg_bounds, seg_slots=seg_slots,
    )
    return plan, per_core


def _build(plan, n_out, n_cores):
    """Trace the Bass program. Returns nc."""
    nc = bacc.Bacc("TRN2", target_bir_lowering=False, debug=False)

    K = plan["K"]
    acc_rows, acc_total = plan["acc_rows"], plan["acc_total"]
    total_slots = plan["total_slots"]
    tile_k = plan["tile_k"]
    seg_bounds = plan["seg_bounds"]
    nseg = len(seg_bounds) - 1
    Cout = 64

    slabt = nc.dram_tensor("slabt", [64, total_slots], BF16,
                           kind="ExternalInput")
    wt = nc.dram_tensor("wt", [64, K * Cout], BF16, kind="ExternalInput")
    sidx = nc.dram_tensor("sidx", [128, total_slots // 16], I16,
                          kind="ExternalInput")
    gb = nc.dram_tensor("gb", [2, Cout], F32, kind="ExternalInput")
    acc0 = nc.dram_tensor("acc0", [acc_total, Cout], F32)
    acc1 = nc.dram_tensor("acc1", [acc_total, Cout], F32)
    accs = [acc0, acc1]
    cc_in = nc.dram_tensor("cc_in", [2, Cout], F32)
    cc_out = nc.dram_tensor("cc_out", [2, Cout], F32, addr_space="Shared")
    y = nc.dram_tensor("y", [acc_rows, Cout], F32, kind="ExternalOutput")

    Tb = acc_rows // 128  # BN column tiles

    # super-segment layout: SUPER_SEGS segments per slab load
    supers = []
    s = 0
    while s < nseg:
        e = min(s + SUPER_SEGS, nseg)
        supers.append((s, e))
        s = e

    with tile.TileContext(nc) as tc:
        with (
            tc.tile_pool(name="const", bufs=1) as cpool,
            tc.tile_pool(name="slab", bufs=3) as slabpool,
            tc.tile_pool(name="oslab", bufs=8) as opool,
            tc.tile_pool(name="sixp", bufs=3) as sixpool,
            tc.tile_pool(name="psum", bufs=8, space="PSUM") as pspool,
        ):
            w_sb = cpool.tile([64, K * Cout], BF16, tag="w")
            nc.sync.dma_start(out=w_sb[:, :], in_=wt[:, :])
            zed = cpool.tile([128, 3200], F32, tag="zed")
            nc.vector.memset(zed[:, :], 0.0)
            zrows = 128 * 3200 // Cout  # 6400 rows per DMA
            # zero acc0 first (gates the first scatter); acc1 zeros are
            # emitted after so they overlap the first super's compute
            for r0 in range(0, acc_total, zrows):
                rcnt = min(zrows, acc_total - r0)
                nc.sync.dma_start(
                    out=acc0[r0:r0 + rcnt, :],
                    in_=zed[:, :rcnt * Cout // 128],
                )

            first = True
            for (s0seg, s1seg) in supers:
                a = seg_bounds[s0seg]
                b = seg_bounds[s1seg]
                ns_sup = b - a
                g = slabpool.tile([64, SUPER_SEGS * plan["seg_slots"]],
                                  BF16, tag="g")
                nc.sync.dma_start(out=g[:, :ns_sup], in_=slabt[:, a:b])
                si_t = sixpool.tile(
                    [128, SUPER_SEGS * plan["seg_slots"] // 16], I16,
                    tag="si")
                nc.sync.dma_start(
                    out=si_t[:, :ns_sup // 16],
                    in_=sidx[:, a // 16:b // 16],
                )
                if first:
                    # overlap acc1 zero-init with the first super
                    for r0 in range(0, acc_total, zrows):
                        rcnt = min(zrows, acc_total - r0)
                        nc.sync.dma_start(
                            out=acc1[r0:r0 + rcnt, :],
                            in_=zed[:, :rcnt * Cout // 128],
                        )
                    first = False
                for seg in range(s0seg, s1seg):
                    sa = seg_bounds[seg]
                    sb = seg_bounds[seg + 1]
                    ns = sb - sa
                    ntile = ns // 128
                    oslab = opool.tile([128, SEG_TILES, Cout], F32,
                                       tag="oslab")
                    for t in range(ntile):
                        col = (sa - a) + t * 128
                        k = int(tile_k[sa // 128 + t])
                        ps = pspool.tile([128, Cout], F32, tag="ps")
                        nc.tensor.matmul(
                            out=ps[:, :],
                            lhsT=g[:, col:col + 128],
                            rhs=w_sb[:, k * Cout:(k + 1) * Cout],
                            start=True, stop=True,
                        )
                        if t % 2 == 0:
                            nc.vector.tensor_copy(
                                out=oslab[:, t, :], in_=ps[:, :])
                        else:
                            nc.scalar.activation(
                                out=oslab[:, t, :], in_=ps[:, :],
                                func=mybir.ActivationFunctionType.Copy)
                    nc.gpsimd.dma_scatter_add(
                        out_ap=accs[seg % 2][:, :],
                        in_ap=oslab[:, :ntile, :],
                        idxs_ap=si_t[:, (sa - a) // 16:(sb - a) // 16],
                        num_idxs=ns,
                        num_idxs_reg=ns,
                        elem_size=Cout,
                    )

        # ---- BN phase ----
        with (
            tc.tile_pool(name="bn", bufs=1) as bnpool,
            tc.tile_pool(name="bns", bufs=4) as bnspool,
            tc.tile_pool(name="bnp", bufs=2, space="PSUM") as bnps,
        ):
            out_sb = bnpool.tile([128, Tb, 64], F32, tag="outsb")
            nc.sync.dma_start(out=out_sb[:, :, :], in_=acc0[0:acc_rows, :])
            bank_sb = bnpool.tile([128, Tb, 64], F32, tag="bank")
            nc.sync.dma_start(out=bank_sb[:, :, :], in_=acc1[0:acc_rows, :])
            nc.vector.tensor_tensor(
                out=out_sb[:, :, :], in0=out_sb[:, :, :],
                in1=bank_sb[:, :, :], op=mybir.AluOpType.add)
            ones = bnpool.tile([128, 1], F32, tag="ones")
            nc.vector.memset(ones[:, :], 1.0)
            sum_ps = bnps.tile([1, 64], F32, tag="sum")
            sq_ps = bnps.tile([1, 64], F32, tag="sq")
            for t in range(Tb):
                nc.tensor.matmul(
                    out=sum_ps[:, :], lhsT=ones[:, :], rhs=out_sb[:, t, :],
                    start=(t == 0), stop=(t == Tb - 1),
                )
            sqt = bnspool.tile([128, 64], F32, tag="sqt")
            for t in range(Tb):
                nc.vector.tensor_tensor(
                    out=sqt[:, :], in0=out_sb[:, t, :], in1=out_sb[:, t, :],
                    op=mybir.AluOpType.mult)
                nc.tensor.matmul(
                    out=sq_ps[:, :], lhsT=ones[:, :], rhs=sqt[:, :],
                    start=(t == 0), stop=(t == Tb - 1),
                )
            st0 = bnspool.tile([1, 64], F32, tag="st0")
            st1 = bnspool.tile([1, 64], F32, tag="st1")
            nc.vector.tensor_copy(out=st0[:, :], in_=sum_ps[:, :])
            nc.vector.tensor_copy(out=st1[:, :], in_=sq_ps[:, :])
            nc.sync.dma_start(out=cc_in[0:1, :], in_=st0[:, :])
            nc.sync.dma_start(out=cc_in[1:2, :], in_=st1[:, :])
            nc.gpsimd.collective_compute(
                "AllReduce",
                mybir.AluOpType.add,
                ins=[cc_in[:, :]],
                outs=[cc_out[:, :]],
                replica_groups=[list(range(n_cores))],
            )
            gs0 = bnspool.tile([1, 64], F32, tag="gs0")
            gs1 = bnspool.tile([1, 64], F32, tag="gs1")
            nc.sync.dma_start(out=gs0[:, :], in_=cc_out[0:1, :])
            nc.sync.dma_start(out=gs1[:, :], in_=cc_out[1:2, :])
            gam_t = bnspool.tile([1, 64], F32, tag="gam")
            bet_t = bnspool.tile([1, 64], F32, tag="bet")
            nc.sync.dma_start(out=gam_t[:, :], in_=gb[0:1, :])
            nc.sync.dma_start(out=bet_t[:, :], in_=gb[1:2, :])

            inv_n = 1.0 / float(n_out)
            mean_t = bnspool.tile([1, 64], F32, tag="mean")
            ex2_t = bnspool.tile([1, 64], F32, tag="ex2")
            var_t = bnspool.tile([1, 64], F32, tag="var")
            sd_t = bnspool.tile([1, 64], F32, tag="sd")
            rs_t = bnspool.tile([1, 64], F32, tag="rs")
            a_t = bnspool.tile([1, 64], F32, tag="a")
            b_t = bnspool.tile([1, 64], F32, tag="b")
            nc.vector.tensor_scalar_mul(mean_t[:, :], gs0[:, :], inv_n)
            nc.vector.tensor_scalar_mul(ex2_t[:, :], gs1[:, :], inv_n)
            nc.vector.tensor_tensor(
                out=var_t[:, :], in0=mean_t[:, :], in1=mean_t[:, :],
                op=mybir.AluOpType.mult)
            nc.vector.tensor_tensor(
                out=var_t[:, :], in0=ex2_t[:, :], in1=var_t[:, :],
                op=mybir.AluOpType.subtract)
            nc.vector.tensor_scalar_add(var_t[:, :], var_t[:, :], BN_EPS)
            nc.scalar.activation(
                out=sd_t[:, :], in_=var_t[:, :],
                func=mybir.ActivationFunctionType.Sqrt)
            nc.vector.reciprocal(out=rs_t[:, :], in_=sd_t[:, :])
            nc.vector.tensor_tensor(
                out=a_t[:, :], in0=gam_t[:, :], in1=rs_t[:, :],
                op=mybir.AluOpType.mult)
            nc.vector.tensor_tensor(
                out=b_t[:, :], in0=mean_t[:, :], in1=a_t[:, :],
                op=mybir.AluOpType.mult)
            nc.vector.tensor_tensor(
                out=b_t[:, :], in0=bet_t[:, :], in1=b_t[:, :],
                op=mybir.AluOpType.subtract)
            # broadcast [1,64] -> [128,64] via PE (ones[1,128]^T @ row)
            ones_row = bnspool.tile([1, 128], F32, tag="ones_row")
            nc.vector.memset(ones_row[:, :], 1.0)
            a_full = bnspool.tile([128, 64], F32, tag="afull")
            b_full = bnspool.tile([128, 64], F32, tag="bfull")
            ab_ps = bnps.tile([128, 64], F32, tag="abps")
            nc.tensor.matmul(
                out=ab_ps[:, :], lhsT=ones_row[:, :], rhs=a_t[:, :],
                start=True, stop=True)
            nc.vector.tensor_copy(out=a_full[:, :], in_=ab_ps[:, :])
            nc.tensor.matmul(
                out=ab_ps[:, :], lhsT=ones_row[:, :], rhs=b_t[:, :],
                start=True, stop=True)
            nc.vector.tensor_copy(out=b_full[:, :], in_=ab_ps[:, :])
            for t in range(Tb):
                nc.vector.tensor_tensor(
                    out=out_sb[:, t, :], in0=out_sb[:, t, :], in1=a_full[:, :],
                    op=mybir.AluOpType.mult)
                nc.vector.tensor_tensor(
                    out=out_sb[:, t, :], in0=out_sb[:, t, :], in1=b_full[:, :],
                    op=mybir.AluOpType.add)
                nc.scalar.activation(
                    out=out_sb[:, t, :], in_=out_sb[:, t, :],
                    func=mybir.ActivationFunctionType.Relu)
            nc.sync.dma_start(out=y[:, :], in_=out_sb[:, :, :])

    nc.compile()
    return nc


def _pack_sidx(svals):
    """[total_slots] int -> [128, total_slots//16] int16 wrapped/tiled."""
    cols = svals.reshape(-1, 16).T.astype(np.int16)  # [16, n/16]
    return np.tile(cols, (8, 1))


def _prepare(feats, W, gamma, beta, in_map, out_map, n_out, n_cores=8,
             *_ignored):
    """Host prep shared by kernel() and tests. Returns (nc, in_maps, plan)."""
    n_out = int(n_out)
    K, Cin, Cout = W.shape
    assert Cin == 64 and Cout == 64
    in_map = np.asarray(in_map, dtype=np.int64)
    out_map = np.asarray(out_map, dtype=np.int64)
    feats = np.asarray(feats, dtype=np.float32)
    W = np.asarray(W, dtype=np.float32)

    plan, per_core = _route(in_map, out_map, n_out, n_cores)

    featsT = np.ascontiguousarray(
        feats.T.astype(ml_dtypes.bfloat16))          # [64, N_in]
    featsT_pad = np.concatenate(
        [featsT, np.zeros((64, 1), dtype=ml_dtypes.bfloat16)], axis=1)

    wt = np.ascontiguousarray(
        W.transpose(1, 0, 2).reshape(64, K * 64).astype(ml_dtypes.bfloat16))

    gb = np.stack([np.asarray(gamma, np.float32),
                   np.asarray(beta, np.float32)])

    nc = _build(plan, n_out, n_cores)
    in_maps = []
    for c in range(n_cores):
        gvals, svals, _ = per_core[c]
        slabt = featsT_pad[:, gvals]                 # -1 -> zero column
        in_maps.append(dict(slabt=np.ascontiguousarray(slabt), wt=wt,
                            sidx=_pack_sidx(svals), gb=gb))
    return nc, in_maps, plan


def kernel(feats, W, gamma, beta, in_map, out_map, n_out):
    from concourse.bass_utils import run_bass_kernel_spmd

    n_cores = 8
    nc, in_maps, plan = _prepare(
        feats, W, gamma, beta, in_map, out_map, n_out, n_cores)
    res = run_bass_kernel_spmd(nc, in_maps, list(range(n_cores)))
    rows = plan["rows_per_core"]
    out = np.concatenate(
        [res.results[c]["y"][:rows] for c in range(n_cores)], axis=0)
    return out.astype(np.float32)


# revision 11
# speedup vs baseline: 2.7433x; 1.0810x over previous
"""Trainium2 Bass kernel for nn_BasicDeconvolutionBlock.

Reference computation:
    gathered = feats[in_map]                         # [K, M, Cin]
    contrib  = einsum('kmc,kcd->kmd', gathered, W)   # [K, M, Cout]
    out      = zeros([n_out, Cout]).at[out_map].add(contrib)
    y        = relu(batchnorm(out))                  # batch stats over n_out rows

Strategy (8 NeuronCores, SPMD):
  - Host routes each (k, m) pair to the core owning its output row
    (row blocks of n_out/8) and lowers the gather to im2col: a per-core
    channel-major slab slabT[64, slots] (bf16) holding feats rows in
    k-major slot order, streamed to SBUF with large contiguous DMAs.
  - GEMM: per 128-slot tile (single k per tile), matmul(lhsT=slab tile
    [64ch,128slots], rhs=W[k][64ch,64]) -> PSUM [128slots,64] f32;
    PSUM->SBUF copies alternate between DVE and Activation engines.
  - Scatter: gpsimd dma_scatter_add (CCE-add, int16 idx) accumulates
    contributions into one of two HBM banks, alternating PER SEGMENT so
    adjacent calls have no WAW hazard and fully overlap (desc-gen of
    call i+1 runs during the DMA of call i).  Duplicate output rows
    inside one call race in hardware, so the host spaces a row's
    occurrences ~cnt/h apart in the slot order and swap-fixes the few
    residual in-segment duplicates.  Calls are capped at 896 indices
    (SWDGE Q7 descriptor-ring limit; larger calls wedge the device).
  - BN: fold banks, ones-matmul row sums + sum of squares, [2,64]
    AllReduce across the 8 cores, normalize + ReLU on chip, output
    shard [rows,64] fp32.
"""

import os
import sys

import numpy as np

sys.path.insert(0, "/opt/trn_rl_repo")

import ml_dtypes  # noqa: E402

from concourse import bacc, bass, mybir  # noqa: E402
import concourse.tile as tile  # noqa: E402

BN_EPS = 1e-5
SEG_TILES = 7       # 896 slots per scatter call (SWDGE ring limit)
SUPER_SEGS = 4      # segments per slab/sidx load
F32 = mybir.dt.float32
BF16 = mybir.dt.bfloat16
I16 = mybir.dt.int16


def _roundup(x, m):
    return (x + m - 1) // m * m


def _order_group(rows):
    """Slot order for one (core, k) group: spread a row's occurrences
    ~cnt/h apart so same-call duplicates are rare. Returns a permutation
    of range(len(rows))."""
    n = len(rows)
    if n == 0:
        return np.empty(0, dtype=np.int64)
    order = np.argsort(rows, kind="stable")
    sr = rows[order]
    first = np.ones(n, dtype=bool)
    first[1:] = sr[1:] != sr[:-1]
    grp = np.cumsum(first) - 1                    # rank of unique row
    grp_start = np.maximum.accumulate(np.where(first, np.arange(n), 0))
    occ = np.arange(n) - grp_start                # occurrence index j
    # occurrence count h per element
    cnt_per_grp = np.bincount(grp)
    h = cnt_per_grp[grp]
    nuniq = cnt_per_grp.size
    key = (occ + grp / max(nuniq, 1)) / h
    final = np.argsort(key, kind="stable")
    return order[final]


def _fix_conflicts(svals, gvals, seg_bounds, group_bounds, dump_row):
    """Ensure no duplicate (non-dump) rows within any segment by swapping
    slots within their k-group. svals/gvals modified in place."""
    nslots = len(svals)
    nseg = len(seg_bounds) - 1
    seg_of = np.zeros(nslots, dtype=np.int64)
    for s in range(nseg):
        seg_of[seg_bounds[s]:seg_bounds[s + 1]] = s
    grp_of = np.zeros(nslots, dtype=np.int64)
    for g in range(len(group_bounds) - 1):
        grp_of[group_bounds[g]:group_bounds[g + 1]] = g

    # per-seg row sets
    seg_sets = [set() for _ in range(nseg)]
    conflicts = []
    is_conflict = np.zeros(nslots, dtype=bool)
    for i in range(nslots):
        r = svals[i]
        if r == dump_row:
            continue
        ss = seg_sets[seg_of[i]]
        if r in ss:
            conflicts.append(i)
            is_conflict[i] = True
        else:
            ss.add(r)
    rng = np.random.default_rng(0)
    for i in conflicts:
        r = int(svals[i])
        g = grp_of[i]
        lo, hi = group_bounds[g], group_bounds[g + 1]
        placed = False
        for _ in range(200):
            j = int(rng.integers(lo, hi))
            sj = seg_of[j]
            if sj == seg_of[i] or is_conflict[j]:
                continue
            rj = int(svals[j])
            # after swap: r goes to seg sj, rj comes to seg of i
            if r in seg_sets[sj]:
                continue
            if rj != dump_row:
                if rj == r or rj in seg_sets[seg_of[i]]:
                    continue
            # apply swap
            si = seg_of[i]
            if rj != dump_row:
                seg_sets[sj].discard(rj)
                seg_sets[si].add(rj)
            seg_sets[sj].add(r)
            svals[i], svals[j] = svals[j], svals[i]
            gvals[i], gvals[j] = gvals[j], gvals[i]
            is_conflict[i] = False
            placed = True
            break
        if not placed:
            raise RuntimeError("conflict fix failed")
    return len(conflicts)


def _route(in_map, out_map, n_out, n_cores):
    """Host-side routing. Returns plan + per-core slot arrays
    (gvals: feats row per slot or -1; svals: local out row per slot)."""
    K, M = in_map.shape
    rows_per_core = n_out // n_cores
    assert rows_per_core * n_cores == n_out
    acc_rows = _roundup(rows_per_core, 128)
    dump_row = acc_rows
    acc_total = acc_rows + 128

    in_flat = in_map.ravel().astype(np.int64)
    out_flat = out_map.ravel().astype(np.int64)
    k_idx = np.repeat(np.arange(K, dtype=np.int64), M)
    core = out_flat // rows_per_core
    row_local = (out_flat - core * rows_per_core).astype(np.int64)

    # per (core, k) counts -> shared caps
    counts = np.zeros((n_cores, K), dtype=np.int64)
    np.add.at(counts, (core, k_idx), 1)
    caps = _roundup(counts.max(axis=0), 128)  # [K]
    group_bounds = np.concatenate([[0], np.cumsum(caps)])
    total_slots = int(group_bounds[-1])

    seg_slots = SEG_TILES * 128
    seg_bounds = list(range(0, total_slots, seg_slots)) + [total_slots]
    if seg_bounds[-1] == seg_bounds[-2]:
        seg_bounds.pop()

    per_core = []
    for c in range(n_cores):
        gvals = np.full(total_slots, -1, dtype=np.int64)
        svals = np.full(total_slots, dump_row, dtype=np.int64)
        sel_c = core == c
        for k in range(K):
            sel = np.nonzero(sel_c & (k_idx == k))[0]
            rows_k = row_local[sel]
            perm = _order_group(rows_k)
            g0 = group_bounds[k]
            n = len(sel)
            gvals[g0:g0 + n] = in_flat[sel][perm]
            svals[g0:g0 + n] = rows_k[perm]
        nfix = _fix_conflicts(svals, gvals, seg_bounds, group_bounds,
                              dump_row)
        per_core.append((gvals, svals, nfix))

    # tile -> k map
    ntiles = total_slots // 128
    tile_k = np.zeros(ntiles, dtype=np.int64)
    for k in range(K):
        tile_k[group_bounds[k] // 128:group_bounds[k + 1] // 128] = k

    plan = dict(
        K=K, rows_per_core=rows_per_core, acc_rows=acc_rows,
        acc_total=acc_total, dump_row=dump_row,
        total_slots=total_slots, ntiles=ntiles, tile_k=tile_k,
        seg_bounds=seg_bounds, seg_slots=seg_slots,
    )
    return plan, per_core


def _build(plan, n_out, n_cores):
    """Trace the Bass program. Returns nc."""
    nc = bacc.Bacc("TRN2", target_bir_lowering=False, debug=False)

    K = plan["K"]
    acc_rows, acc_total = plan["acc_rows"], plan["acc_total"]
    total_slots = plan["total_slots"]
    tile_k = plan["tile_k"]
    seg_bounds = plan["seg_bounds"]
    nseg = len(seg_bounds) - 1
    Cout = 64

    slabt = nc.dram_tensor("slabt", [64, total_slots], BF16,
                           kind="ExternalInput")
    wt = nc.dram_tensor("wt", [64, K * Cout], BF16, kind="ExternalInput")
    sidx = nc.dram_tensor("sidx", [128, total_slots // 16], I16,
                          kind="ExternalInput")
    gb = nc.dram_tensor("gb", [2, Cout], F32, kind="ExternalInput")
    # bf16 accumulator banks, rows padded to 128 cols so the scatter's
    # 256B row stride holds (elem_step=128, payload=64 cols)
    acc0 = nc.dram_tensor("acc0", [acc_total, 128], BF16)
    acc1 = nc.dram_tensor("acc1", [acc_total, 128], BF16)
    accs = [acc0, acc1]
    cc_in = nc.dram_tensor("cc_in", [2, Cout], F32)
    cc_out = nc.dram_tensor("cc_out", [2, Cout], F32, addr_space="Shared")
    y = nc.dram_tensor("y", [acc_rows, Cout], F32, kind="ExternalOutput")

    Tb = acc_rows // 128  # BN column tiles

    # super-segment layout: SUPER_SEGS segments per slab load
    supers = []
    s = 0
    while s < nseg:
        e = min(s + SUPER_SEGS, nseg)
        supers.append((s, e))
        s = e

    with tile.TileContext(nc) as tc:
        with (
            tc.tile_pool(name="const", bufs=1) as cpool,
            tc.tile_pool(name="slab", bufs=3) as slabpool,
            tc.tile_pool(name="oslab", bufs=8) as opool,
            tc.tile_pool(name="sixp", bufs=3) as sixpool,
            tc.tile_pool(name="psum", bufs=8, space="PSUM") as pspool,
        ):
            w_sb = cpool.tile([64, K * Cout], BF16, tag="w")
            nc.sync.dma_start(out=w_sb[:, :], in_=wt[:, :])
            zed = cpool.tile([128, 6400], BF16, tag="zed")
            nc.vector.memset(zed[:, :], 0.0)
            zrows = 128 * 6400 // 128  # 6400 rows per DMA
            # zero acc0 first (gates the first scatter); acc1 zeros are
            # emitted after so they overlap the first super's compute
            for r0 in range(0, acc_total, zrows):
                rcnt = min(zrows, acc_total - r0)
                nc.sync.dma_start(
                    out=acc0[r0:r0 + rcnt, :],
                    in_=zed[:, :rcnt],
                )

            first = True
            for (s0seg, s1seg) in supers:
                a = seg_bounds[s0seg]
                b = seg_bounds[s1seg]
                ns_sup = b - a
                g = slabpool.tile([64, SUPER_SEGS * plan["seg_slots"]],
                                  BF16, tag="g")
                nc.sync.dma_start(out=g[:, :ns_sup], in_=slabt[:, a:b])
                si_t = sixpool.tile(
                    [128, SUPER_SEGS * plan["seg_slots"] // 16], I16,
                    tag="si")
                nc.sync.dma_start(
                    out=si_t[:, :ns_sup // 16],
                    in_=sidx[:, a // 16:b // 16],
                )
                if first:
                    # overlap acc1 zero-init with the first super
                    for r0 in range(0, acc_total, zrows):
                        rcnt = min(zrows, acc_total - r0)
                        nc.sync.dma_start(
                            out=acc1[r0:r0 + rcnt, :],
                            in_=zed[:, :rcnt],
                        )
                    first = False
                for seg in range(s0seg, s1seg):
                    sa = seg_bounds[seg]
                    sb = seg_bounds[seg + 1]
                    ns = sb - sa
                    ntile = ns // 128
                    oslab = opool.tile([128, SEG_TILES, Cout], BF16,
                                       tag="oslab")
                    for t in range(ntile):
                        col = (sa - a) + t * 128
                        k = int(tile_k[sa // 128 + t])
                        ps = pspool.tile([128, Cout], F32, tag="ps")
                        nc.tensor.matmul(
                            out=ps[:, :],
                            lhsT=g[:, col:col + 128],
                            rhs=w_sb[:, k * Cout:(k + 1) * Cout],
                            start=True, stop=True,
                        )
                        if t % 2 == 0:
                            nc.vector.tensor_copy(
                                out=oslab[:, t, :], in_=ps[:, :])
                        else:
                            nc.scalar.activation(
                                out=oslab[:, t, :], in_=ps[:, :],
                                func=mybir.ActivationFunctionType.Copy)
                    nc.gpsimd.dma_scatter_add(
                        out_ap=accs[seg % 2][:, 0:Cout],
                        in_ap=oslab[:, :ntile, :],
                        idxs_ap=si_t[:, (sa - a) // 16:(sb - a) // 16],
                        num_idxs=ns,
                        num_idxs_reg=ns,
                        elem_size=Cout,
                        elem_step=128,
                    )

        # ---- BN phase ----
        with (
            tc.tile_pool(name="bn", bufs=1) as bnpool,
            tc.tile_pool(name="bns", bufs=4) as bnspool,
            tc.tile_pool(name="bnp", bufs=2, space="PSUM") as bnps,
        ):
            a0_sb = bnpool.tile([128, Tb, 128], BF16, tag="a0sb")
            nc.sync.dma_start(out=a0_sb[:, :, :], in_=acc0[0:acc_rows, :])
            a1_sb = bnpool.tile([128, Tb, 128], BF16, tag="a1sb")
            nc.sync.dma_start(out=a1_sb[:, :, :], in_=acc1[0:acc_rows, :])
            out_sb = bnpool.tile([128, Tb, 64], F32, tag="outsb")
            nc.vector.tensor_tensor(
                out=out_sb[:, :, :], in0=a0_sb[:, :, 0:64],
                in1=a1_sb[:, :, 0:64], op=mybir.AluOpType.add)
            ones = bnpool.tile([128, 1], F32, tag="ones")
            nc.vector.memset(ones[:, :], 1.0)
            sum_ps = bnps.tile([1, 64], F32, tag="sum")
            sq_ps = bnps.tile([1, 64], F32, tag="sq")
            for t in range(Tb):
                nc.tensor.matmul(
                    out=sum_ps[:, :], lhsT=ones[:, :], rhs=out_sb[:, t, :],
                    start=(t == 0), stop=(t == Tb - 1),
                )
            sqt = bnspool.tile([128, 64], F32, tag="sqt")
            for t in range(Tb):
                nc.vector.tensor_tensor(
                    out=sqt[:, :], in0=out_sb[:, t, :], in1=out_sb[:, t, :],
                    op=mybir.AluOpType.mult)
                nc.tensor.matmul(
                    out=sq_ps[:, :], lhsT=ones[:, :], rhs=sqt[:, :],
                    start=(t == 0), stop=(t == Tb - 1),
                )
            st0 = bnspool.tile([1, 64], F32, tag="st0")
            st1 = bnspool.tile([1, 64], F32, tag="st1")
            nc.vector.tensor_copy(out=st0[:, :], in_=sum_ps[:, :])
            nc.vector.tensor_copy(out=st1[:, :], in_=sq_ps[:, :])
            nc.sync.dma_start(out=cc_in[0:1, :], in_=st0[:, :])
            nc.sync.dma_start(out=cc_in[1:2, :], in_=st1[:, :])
            nc.gpsimd.collective_compute(
                "AllReduce",
                mybir.AluOpType.add,
                ins=[cc_in[:, :]],
                outs=[cc_out[:, :]],
                replica_groups=[list(range(n_cores))],
            )
            gs0 = bnspool.tile([1, 64], F32, tag="gs0")
            gs1 = bnspool.tile([1, 64], F32, tag="gs1")
            nc.sync.dma_start(out=gs0[:, :], in_=cc_out[0:1, :])
            nc.sync.dma_start(out=gs1[:, :], in_=cc_out[1:2, :])
            gam_t = bnspool.tile([1, 64], F32, tag="gam")
            bet_t = bnspool.tile([1, 64], F32, tag="bet")
            nc.sync.dma_start(out=gam_t[:, :], in_=gb[0:1, :])
            nc.sync.dma_start(out=bet_t[:, :], in_=gb[1:2, :])

            inv_n = 1.0 / float(n_out)
            mean_t = bnspool.tile([1, 64], F32, tag="mean")
            ex2_t = bnspool.tile([1, 64], F32, tag="ex2")
            var_t = bnspool.tile([1, 64], F32, tag="var")
            sd_t = bnspool.tile([1, 64], F32, tag="sd")
            rs_t = bnspool.tile([1, 64], F32, tag="rs")
            a_t = bnspool.tile([1, 64], F32, tag="a")
            b_t = bnspool.tile([1, 64], F32, tag="b")
            nc.vector.tensor_scalar_mul(mean_t[:, :], gs0[:, :], inv_n)
            nc.vector.tensor_scalar_mul(ex2_t[:, :], gs1[:, :], inv_n)
            nc.vector.tensor_tensor(
                out=var_t[:, :], in0=mean_t[:, :], in1=mean_t[:, :],
                op=mybir.AluOpType.mult)
            nc.vector.tensor_tensor(
                out=var_t[:, :], in0=ex2_t[:, :], in1=var_t[:, :],
                op=mybir.AluOpType.subtract)
            nc.vector.tensor_scalar_add(var_t[:, :], var_t[:, :], BN_EPS)
            nc.scalar.activation(
                out=sd_t[:, :], in_=var_t[:, :],
                func=mybir.ActivationFunctionType.Sqrt)
            nc.vector.reciprocal(out=rs_t[:, :], in_=sd_t[:, :])
            nc.vector.tensor_tensor(
                out=a_t[:, :], in0=gam_t[:, :], in1=rs_t[:, :],
                op=mybir.AluOpType.mult)
            nc.vector.tensor_tensor(
                out=b_t[:, :], in0=mean_t[:, :], in1=a_t[:, :],
                op=mybir.AluOpType.mult)
            nc.vector.tensor_tensor(
                out=b_t[:, :], in0=bet_t[:, :], in1=b_t[:, :],
                op=mybir.AluOpType.subtract)
            # broadcast [1,64] -> [128,64] via PE (ones[1,128]^T @ row)
            ones_row = bnspool.tile([1, 128], F32, tag="ones_row")
            nc.vector.memset(ones_row[:, :], 1.0)
            a_full = bnspool.tile([128, 64], F32, tag="afull")
            b_full = bnspool.tile([128, 64], F32, tag="bfull")
            ab_ps = bnps.tile([128, 64], F32, tag="abps")
            nc.tensor.matmul(
                out=ab_ps[:, :], lhsT=ones_row[:, :], rhs=a_t[:, :],
                start=True, stop=True)
            nc.vector.tensor_copy(out=a_full[:, :], in_=ab_ps[:, :])
            nc.tensor.matmul(
                out=ab_ps[:, :], lhsT=ones_row[:, :], rhs=b_t[:, :],
                start=True, stop=True)
            nc.vector.tensor_copy(out=b_full[:, :], in_=ab_ps[:, :])
            for t in range(Tb):
                nc.vector.tensor_tensor(
                    out=out_sb[:, t, :], in0=out_sb[:, t, :], in1=a_full[:, :],
                    op=mybir.AluOpType.mult)
                nc.vector.tensor_tensor(
                    out=out_sb[:, t, :], in0=out_sb[:, t, :], in1=b_full[:, :],
                    op=mybir.AluOpType.add)
                nc.scalar.activation(
                    out=out_sb[:, t, :], in_=out_sb[:, t, :],
                    func=mybir.ActivationFunctionType.Relu)
            nc.sync.dma_start(out=y[:, :], in_=out_sb[:, :, :])

    nc.compile()
    return nc


def _pack_sidx(svals):
    """[total_slots] int -> [128, total_slots//16] int16 wrapped/tiled."""
    cols = svals.reshape(-1, 16).T.astype(np.int16)  # [16, n/16]
    return np.tile(cols, (8, 1))


def _prepare(feats, W, gamma, beta, in_map, out_map, n_out, n_cores=8,
             *_ignored):
    """Host prep shared by kernel() and tests. Returns (nc, in_maps, plan)."""
    n_out = int(n_out)
    K, Cin, Cout = W.shape
    assert Cin == 64 and Cout == 64
    in_map = np.asarray(in_map, dtype=np.int64)
    out_map = np.asarray(out_map, dtype=np.int64)
    feats = np.asarray(feats, dtype=np.float32)
    W = np.asarray(W, dtype=np.float32)

    plan, per_core = _route(in_map, out_map, n_out, n_cores)

    featsT = np.ascontiguousarray(
        feats.T.astype(ml_dtypes.bfloat16))          # [64, N_in]
    featsT_pad = np.concatenate(
        [featsT, np.zeros((64, 1), dtype=ml_dtypes.bfloat16)], axis=1)

    wt = np.ascontiguousarray(
        W.transpose(1, 0, 2).reshape(64, K * 64).astype(ml_dtypes.bfloat16))

    gb = np.stack([np.asarray(gamma, np.float32),
                   np.asarray(beta, np.float32)])

    nc = _build(plan, n_out, n_cores)
    in_maps = []
    for c in range(n_cores):
        gvals, svals, _ = per_core[c]
        slabt = featsT_pad[:, gvals]                 # -1 -> zero column
        in_maps.append(dict(slabt=np.ascontiguousarray(slabt), wt=wt,
                            sidx=_pack_sidx(svals), gb=gb))
    return nc, in_maps, plan


def kernel(feats, W, gamma, beta, in_map, out_map, n_out):
    from concourse.bass_utils import run_bass_kernel_spmd

    n_cores = 8
    nc, in_maps, plan = _prepare(
        feats, W, gamma, beta, in_map, out_map, n_out, n_cores)
    res = run_bass_kernel_spmd(nc, in_maps, list(range(n_cores)))
    rows = plan["rows_per_core"]
    out = np.concatenate(
        [res.results[c]["y"][:rows] for c in range(n_cores)], axis=0)
    return out.astype(np.float32)


# revision 21
# speedup vs baseline: 2.9264x; 1.0668x over previous
"""Trainium2 Bass kernel for nn_BasicDeconvolutionBlock.

Reference computation:
    gathered = feats[in_map]                         # [K, M, Cin]
    contrib  = einsum('kmc,kcd->kmd', gathered, W)   # [K, M, Cout]
    out      = zeros([n_out, Cout]).at[out_map].add(contrib)
    y        = relu(batchnorm(out))                  # batch stats over n_out rows

Strategy (8 NeuronCores, SPMD):
  - Host routes each (k, m) pair to the core owning its output row
    (row blocks of n_out/8) and lowers the gather to im2col: a per-core
    channel-major slab slabT[64, slots] (bf16) holding feats rows in
    k-major slot order, streamed to SBUF with large contiguous DMAs.
  - GEMM: per 128-slot tile (single k per tile), matmul(lhsT=slab tile
    [64ch,128slots], rhs=W[k][64ch,64]) -> PSUM [128slots,64] f32;
    PSUM->SBUF copies alternate between DVE and Activation engines.
  - Scatter: gpsimd dma_scatter_add (CCE-add, int16 idx) accumulates
    contributions into one of two HBM banks, alternating PER SEGMENT so
    adjacent calls have no WAW hazard and fully overlap (desc-gen of
    call i+1 runs during the DMA of call i).  Duplicate output rows
    inside one call race in hardware, so the host spaces a row's
    occurrences ~cnt/h apart in the slot order and swap-fixes the few
    residual in-segment duplicates.  Calls are capped at 896 indices
    (SWDGE Q7 descriptor-ring limit; larger calls wedge the device).
  - BN: fold banks, ones-matmul row sums + sum of squares, [2,64]
    AllReduce across the 8 cores, normalize + ReLU on chip, output
    shard [rows,64] fp32.
"""

import os
import sys

import numpy as np

sys.path.insert(0, "/opt/trn_rl_repo")

import ml_dtypes  # noqa: E402

from concourse import bacc, bass, mybir  # noqa: E402
import concourse.tile as tile  # noqa: E402

BN_EPS = 1e-5
SEG_TILES = int(os.environ.get("DECONV_SEG_TILES", "7"))
SUPER_SEGS = int(os.environ.get("DECONV_SUPER_SEGS", "4"))
NBANKS = int(os.environ.get("DECONV_NBANKS", "3"))
# SWDGE descriptor-ring carveout; ring must hold the in-flight scatter
# calls (NBANKS x SEG_TILES x 128 descriptors)
DMA_SCRATCH = int(os.environ.get("DECONV_SCRATCH", "16384"))
F32 = mybir.dt.float32
BF16 = mybir.dt.bfloat16
I16 = mybir.dt.int16


def _roundup(x, m):
    return (x + m - 1) // m * m


def _order_group(rows):
    """Slot order for one (core, k) group: spread a row's occurrences
    ~cnt/h apart so same-call duplicates are rare. Returns a permutation
    of range(len(rows))."""
    n = len(rows)
    if n == 0:
        return np.empty(0, dtype=np.int64)
    order = np.argsort(rows, kind="stable")
    sr = rows[order]
    first = np.ones(n, dtype=bool)
    first[1:] = sr[1:] != sr[:-1]
    grp = np.cumsum(first) - 1                    # rank of unique row
    grp_start = np.maximum.accumulate(np.where(first, np.arange(n), 0))
    occ = np.arange(n) - grp_start                # occurrence index j
    # occurrence count h per element
    cnt_per_grp = np.bincount(grp)
    h = cnt_per_grp[grp]
    nuniq = cnt_per_grp.size
    key = (occ + grp / max(nuniq, 1)) / h
    final = np.argsort(key, kind="stable")
    return order[final]


def _fix_conflicts(svals, gvals, seg_bounds, group_bounds, dump_row):
    """Ensure no duplicate (non-dump) rows within any segment by swapping
    slots within their k-group. svals/gvals modified in place."""
    nslots = len(svals)
    nseg = len(seg_bounds) - 1
    seg_of = np.zeros(nslots, dtype=np.int64)
    for s in range(nseg):
        seg_of[seg_bounds[s]:seg_bounds[s + 1]] = s
    grp_of = np.zeros(nslots, dtype=np.int64)
    for g in range(len(group_bounds) - 1):
        grp_of[group_bounds[g]:group_bounds[g + 1]] = g

    # per-seg row sets
    seg_sets = [set() for _ in range(nseg)]
    conflicts = []
    is_conflict = np.zeros(nslots, dtype=bool)
    for i in range(nslots):
        r = svals[i]
        if r == dump_row:
            continue
        ss = seg_sets[seg_of[i]]
        if r in ss:
            conflicts.append(i)
            is_conflict[i] = True
        else:
            ss.add(r)
    rng = np.random.default_rng(0)
    for i in conflicts:
        r = int(svals[i])
        g = grp_of[i]
        lo, hi = group_bounds[g], group_bounds[g + 1]
        placed = False
        cands = list(rng.integers(lo, hi, size=200)) + list(range(lo, hi))
        for j in cands:
            j = int(j)
            sj = seg_of[j]
            if sj == seg_of[i] or is_conflict[j]:
                continue
            rj = int(svals[j])
            # after swap: r goes to seg sj, rj comes to seg of i
            if r in seg_sets[sj]:
                continue
            if rj != dump_row:
                if rj == r or rj in seg_sets[seg_of[i]]:
                    continue
            # apply swap
            si = seg_of[i]
            if rj != dump_row:
                seg_sets[sj].discard(rj)
                seg_sets[si].add(rj)
            seg_sets[sj].add(r)
            svals[i], svals[j] = svals[j], svals[i]
            gvals[i], gvals[j] = gvals[j], gvals[i]
            is_conflict[i] = False
            placed = True
            break
        if not placed:
            if os.environ.get("DECONV_ALLOW_SPILL", "0") == "1":
                svals[i] = dump_row  # drops the contribution (test only)
                continue
            raise RuntimeError("conflict fix failed")
    return len(conflicts)


def _route(in_map, out_map, n_out, n_cores):
    """Host-side routing. Returns plan + per-core slot arrays
    (gvals: feats row per slot or -1; svals: local out row per slot)."""
    K, M = in_map.shape
    rows_per_core = n_out // n_cores
    assert rows_per_core * n_cores == n_out
    acc_rows = _roundup(rows_per_core, 128)
    dump_row = acc_rows
    acc_total = acc_rows + 128

    in_flat = in_map.ravel().astype(np.int64)
    out_flat = out_map.ravel().astype(np.int64)
    k_idx = np.repeat(np.arange(K, dtype=np.int64), M)
    core = out_flat // rows_per_core
    row_local = (out_flat - core * rows_per_core).astype(np.int64)

    # per (core, k) counts -> shared caps
    counts = np.zeros((n_cores, K), dtype=np.int64)
    np.add.at(counts, (core, k_idx), 1)
    caps = _roundup(counts.max(axis=0), 128)  # [K]
    group_bounds = np.concatenate([[0], np.cumsum(caps)])
    total_slots = int(group_bounds[-1])

    seg_slots = SEG_TILES * 128
    seg_bounds = list(range(0, total_slots, seg_slots)) + [total_slots]
    if seg_bounds[-1] == seg_bounds[-2]:
        seg_bounds.pop()

    per_core = []
    for c in range(n_cores):
        gvals = np.full(total_slots, -1, dtype=np.int64)
        svals = np.full(total_slots, dump_row, dtype=np.int64)
        sel_c = core == c
        for k in range(K):
            sel = np.nonzero(sel_c & (k_idx == k))[0]
            rows_k = row_local[sel]
            perm = _order_group(rows_k)
            g0 = group_bounds[k]
            n = len(sel)
            gvals[g0:g0 + n] = in_flat[sel][perm]
            svals[g0:g0 + n] = rows_k[perm]
        nfix = _fix_conflicts(svals, gvals, seg_bounds, group_bounds,
                              dump_row)
        per_core.append((gvals, svals, nfix))

    # tile -> k map
    ntiles = total_slots // 128
    tile_k = np.zeros(ntiles, dtype=np.int64)
    for k in range(K):
        tile_k[group_bounds[k] // 128:group_bounds[k + 1] // 128] = k

    plan = dict(
        K=K, rows_per_core=rows_per_core, acc_rows=acc_rows,
        acc_total=acc_total, dump_row=dump_row,
        total_slots=total_slots, ntiles=ntiles, tile_k=tile_k,
        seg_bounds=seg_bounds, seg_slots=seg_slots,
    )
    return plan, per_core


def _build(plan, n_out, n_cores):
    """Trace the Bass program. Returns nc."""
    nc = bacc.Bacc("TRN2", target_bir_lowering=False, debug=False,
                   dynamic_dma_scratch_size=DMA_SCRATCH)

    K = plan["K"]
    acc_rows, acc_total = plan["acc_rows"], plan["acc_total"]
    total_slots = plan["total_slots"]
    tile_k = plan["tile_k"]
    seg_bounds = plan["seg_bounds"]
    nseg = len(seg_bounds) - 1
    Cout = 64

    slabt = nc.dram_tensor("slabt", [64, total_slots], BF16,
                           kind="ExternalInput")
    wt = nc.dram_tensor("wt", [64, K * Cout], BF16, kind="ExternalInput")
    sidx = nc.dram_tensor("sidx", [128, total_slots // 16], I16,
                          kind="ExternalInput")
    gb = nc.dram_tensor("gb", [2, Cout], F32, kind="ExternalInput")
    # bf16 accumulator banks, rows padded to 128 cols so the scatter's
    # 256B row stride holds (elem_step=128, payload=64 cols)
    accs = [nc.dram_tensor(f"acc{b}", [acc_total, 128], BF16)
            for b in range(NBANKS)]
    cc_in = nc.dram_tensor("cc_in", [2, Cout], F32)
    cc_out = nc.dram_tensor("cc_out", [2, Cout], F32, addr_space="Shared")
    y = nc.dram_tensor("y", [acc_rows, Cout], F32, kind="ExternalOutput")

    Tb = acc_rows // 128  # BN column tiles

    # super-segment layout: SUPER_SEGS segments per slab load
    supers = []
    s = 0
    while s < nseg:
        e = min(s + SUPER_SEGS, nseg)
        supers.append((s, e))
        s = e

    with tile.TileContext(nc) as tc:
        with (
            tc.tile_pool(name="const", bufs=1) as cpool,
            tc.tile_pool(name="slab", bufs=3) as slabpool,
            tc.tile_pool(name="oslab", bufs=8) as opool,
            tc.tile_pool(name="sixp", bufs=3) as sixpool,
            tc.tile_pool(name="psum", bufs=8, space="PSUM") as pspool,
        ):
            w_sb = cpool.tile([64, K * Cout], BF16, tag="w")
            nc.sync.dma_start(out=w_sb[:, :], in_=wt[:, :])
            zed = cpool.tile([128, 6400], BF16, tag="zed")
            nc.vector.memset(zed[:, :], 0.0)
            zrows = 128 * 6400 // 128  # 6400 rows per DMA
            # zero acc0 first (gates the first scatter); later banks'
            # zeros are emitted after so they overlap the first supers
            for r0 in range(0, acc_total, zrows):
                rcnt = min(zrows, acc_total - r0)
                nc.sync.dma_start(
                    out=accs[0][r0:r0 + rcnt, :],
                    in_=zed[:, :rcnt],
                )

            first = True
            for (s0seg, s1seg) in supers:
                a = seg_bounds[s0seg]
                b = seg_bounds[s1seg]
                ns_sup = b - a
                g = slabpool.tile([64, SUPER_SEGS * plan["seg_slots"]],
                                  BF16, tag="g")
                nc.sync.dma_start(out=g[:, :ns_sup], in_=slabt[:, a:b])
                si_t = sixpool.tile(
                    [128, SUPER_SEGS * plan["seg_slots"] // 16], I16,
                    tag="si")
                nc.sync.dma_start(
                    out=si_t[:, :ns_sup // 16],
                    in_=sidx[:, a // 16:b // 16],
                )
                if first:
                    # overlap remaining banks' zero-init with super 0
                    for bank in accs[1:]:
                        for r0 in range(0, acc_total, zrows):
                            rcnt = min(zrows, acc_total - r0)
                            nc.sync.dma_start(
                                out=bank[r0:r0 + rcnt, :],
                                in_=zed[:, :rcnt],
                            )
                    first = False
                for seg in range(s0seg, s1seg):
                    sa = seg_bounds[seg]
                    sb = seg_bounds[seg + 1]
                    ns = sb - sa
                    ntile = ns // 128
                    oslab = opool.tile([128, SEG_TILES, Cout], BF16,
                                       tag="oslab")
                    for t in range(ntile):
                        col = (sa - a) + t * 128
                        k = int(tile_k[sa // 128 + t])
                        ps = pspool.tile([128, Cout], F32, tag="ps")
                        nc.tensor.matmul(
                            out=ps[:, :],
                            lhsT=g[:, col:col + 128],
                            rhs=w_sb[:, k * Cout:(k + 1) * Cout],
                            start=True, stop=True,
                        )
                        if t % 2 == 0:
                            nc.vector.tensor_copy(
                                out=oslab[:, t, :], in_=ps[:, :])
                        else:
                            nc.scalar.activation(
                                out=oslab[:, t, :], in_=ps[:, :],
                                func=mybir.ActivationFunctionType.Copy)
                    nc.gpsimd.dma_scatter_add(
                        out_ap=accs[seg % NBANKS][:, 0:Cout],
                        in_ap=oslab[:, :ntile, :],
                        idxs_ap=si_t[:, (sa - a) // 16:(sb - a) // 16],
                        num_idxs=ns,
                        num_idxs_reg=ns,
                        elem_size=Cout,
                        elem_step=128,
                    )

        # ---- BN phase ----
        with (
            tc.tile_pool(name="bn", bufs=1) as bnpool,
            tc.tile_pool(name="bns", bufs=4) as bnspool,
            tc.tile_pool(name="bnp", bufs=2, space="PSUM") as bnps,
        ):
            bank_sbs = []
            for b in range(NBANKS):
                bsb = bnpool.tile([128, Tb, 64], BF16, tag=f"a{b}sb")
                nc.sync.dma_start(out=bsb[:, :, :],
                                  in_=accs[b][0:acc_rows, 0:64])
                bank_sbs.append(bsb)
            out_sb = bnpool.tile([128, Tb, 64], F32, tag="outsb")
            nc.vector.tensor_tensor(
                out=out_sb[:, :, :], in0=bank_sbs[0][:, :, :],
                in1=bank_sbs[1][:, :, :], op=mybir.AluOpType.add)
            for b in range(2, NBANKS):
                nc.vector.tensor_tensor(
                    out=out_sb[:, :, :], in0=out_sb[:, :, :],
                    in1=bank_sbs[b][:, :, :], op=mybir.AluOpType.add)
            ones = bnpool.tile([128, 1], F32, tag="ones")
            nc.vector.memset(ones[:, :], 1.0)
            sum_ps = bnps.tile([1, 64], F32, tag="sum")
            sq_ps = bnps.tile([1, 64], F32, tag="sq")
            for t in range(Tb):
                nc.tensor.matmul(
                    out=sum_ps[:, :], lhsT=ones[:, :], rhs=out_sb[:, t, :],
                    start=(t == 0), stop=(t == Tb - 1),
                )
            sqt = bnspool.tile([128, 64], F32, tag="sqt")
            for t in range(Tb):
                nc.vector.tensor_tensor(
                    out=sqt[:, :], in0=out_sb[:, t, :], in1=out_sb[:, t, :],
                    op=mybir.AluOpType.mult)
                nc.tensor.matmul(
                    out=sq_ps[:, :], lhsT=ones[:, :], rhs=sqt[:, :],
                    start=(t == 0), stop=(t == Tb - 1),
                )
            st0 = bnspool.tile([1, 64], F32, tag="st0")
            st1 = bnspool.tile([1, 64], F32, tag="st1")
            nc.vector.tensor_copy(out=st0[:, :], in_=sum_ps[:, :])
            nc.vector.tensor_copy(out=st1[:, :], in_=sq_ps[:, :])
            nc.sync.dma_start(out=cc_in[0:1, :], in_=st0[:, :])
            nc.sync.dma_start(out=cc_in[1:2, :], in_=st1[:, :])
            nc.gpsimd.collective_compute(
                "AllReduce",
                mybir.AluOpType.add,
                ins=[cc_in[:, :]],
                outs=[cc_out[:, :]],
                replica_groups=[list(range(n_cores))],
            )
            gs0 = bnspool.tile([1, 64], F32, tag="gs0")
            gs1 = bnspool.tile([1, 64], F32, tag="gs1")
            nc.sync.dma_start(out=gs0[:, :], in_=cc_out[0:1, :])
            nc.sync.dma_start(out=gs1[:, :], in_=cc_out[1:2, :])
            gam_t = bnspool.tile([1, 64], F32, tag="gam")
            bet_t = bnspool.tile([1, 64], F32, tag="bet")
            nc.sync.dma_start(out=gam_t[:, :], in_=gb[0:1, :])
            nc.sync.dma_start(out=bet_t[:, :], in_=gb[1:2, :])

            inv_n = 1.0 / float(n_out)
            mean_t = bnspool.tile([1, 64], F32, tag="mean")
            ex2_t = bnspool.tile([1, 64], F32, tag="ex2")
            var_t = bnspool.tile([1, 64], F32, tag="var")
            sd_t = bnspool.tile([1, 64], F32, tag="sd")
            rs_t = bnspool.tile([1, 64], F32, tag="rs")
            a_t = bnspool.tile([1, 64], F32, tag="a")
            b_t = bnspool.tile([1, 64], F32, tag="b")
            nc.vector.tensor_scalar_mul(mean_t[:, :], gs0[:, :], inv_n)
            nc.vector.tensor_scalar_mul(ex2_t[:, :], gs1[:, :], inv_n)
            nc.vector.tensor_tensor(
                out=var_t[:, :], in0=mean_t[:, :], in1=mean_t[:, :],
                op=mybir.AluOpType.mult)
            nc.vector.tensor_tensor(
                out=var_t[:, :], in0=ex2_t[:, :], in1=var_t[:, :],
                op=mybir.AluOpType.subtract)
            nc.vector.tensor_scalar_add(var_t[:, :], var_t[:, :], BN_EPS)
            nc.scalar.activation(
                out=sd_t[:, :], in_=var_t[:, :],
                func=mybir.ActivationFunctionType.Sqrt)
            nc.vector.reciprocal(out=rs_t[:, :], in_=sd_t[:, :])
            nc.vector.tensor_tensor(
                out=a_t[:, :], in0=gam_t[:, :], in1=rs_t[:, :],
                op=mybir.AluOpType.mult)
            nc.vector.tensor_tensor(
                out=b_t[:, :], in0=mean_t[:, :], in1=a_t[:, :],
                op=mybir.AluOpType.mult)
            nc.vector.tensor_tensor(
                out=b_t[:, :], in0=bet_t[:, :], in1=b_t[:, :],
                op=mybir.AluOpType.subtract)
            # broadcast [1,64] -> [128,64] via PE (ones[1,128]^T @ row)
            ones_row = bnspool.tile([1, 128], F32, tag="ones_row")
            nc.vector.memset(ones_row[:, :], 1.0)
            a_full = bnspool.tile([128, 64], F32, tag="afull")
            b_full = bnspool.tile([128, 64], F32, tag="bfull")
            ab_ps = bnps.tile([128, 64], F32, tag="abps")
            nc.tensor.matmul(
                out=ab_ps[:, :], lhsT=ones_row[:, :], rhs=a_t[:, :],
                start=True, stop=True)
            nc.vector.tensor_copy(out=a_full[:, :], in_=ab_ps[:, :])
            nc.tensor.matmul(
                out=ab_ps[:, :], lhsT=ones_row[:, :], rhs=b_t[:, :],
                start=True, stop=True)
            nc.vector.tensor_copy(out=b_full[:, :], in_=ab_ps[:, :])
            for t in range(Tb):
                nc.vector.tensor_tensor(
                    out=out_sb[:, t, :], in0=out_sb[:, t, :], in1=a_full[:, :],
                    op=mybir.AluOpType.mult)
                nc.vector.tensor_tensor(
                    out=out_sb[:, t, :], in0=out_sb[:, t, :], in1=b_full[:, :],
                    op=mybir.AluOpType.add)
                nc.scalar.activation(
                    out=out_sb[:, t, :], in_=out_sb[:, t, :],
                    func=mybir.ActivationFunctionType.Relu)
            nc.sync.dma_start(out=y[:, :], in_=out_sb[:, :, :])

    nc.compile()
    return nc


def _pack_sidx(svals):
    """[total_slots] int -> [128, total_slots//16] int16 wrapped/tiled."""
    cols = svals.reshape(-1, 16).T.astype(np.int16)  # [16, n/16]
    return np.tile(cols, (8, 1))


def _prepare(feats, W, gamma, beta, in_map, out_map, n_out, n_cores=8,
             *_ignored):
    """Host prep shared by kernel() and tests. Returns (nc, in_maps, plan)."""
    n_out = int(n_out)
    K, Cin, Cout = W.shape
    assert Cin == 64 and Cout == 64
    in_map = np.asarray(in_map, dtype=np.int64)
    out_map = np.asarray(out_map, dtype=np.int64)
    feats = np.asarray(feats, dtype=np.float32)
    W = np.asarray(W, dtype=np.float32)

    plan, per_core = _route(in_map, out_map, n_out, n_cores)

    featsT = np.ascontiguousarray(
        feats.T.astype(ml_dtypes.bfloat16))          # [64, N_in]
    featsT_pad = np.concatenate(
        [featsT, np.zeros((64, 1), dtype=ml_dtypes.bfloat16)], axis=1)

    wt = np.ascontiguousarray(
        W.transpose(1, 0, 2).reshape(64, K * 64).astype(ml_dtypes.bfloat16))

    gb = np.stack([np.asarray(gamma, np.float32),
                   np.asarray(beta, np.float32)])

    nc = _build(plan, n_out, n_cores)
    in_maps = []
    for c in range(n_cores):
        gvals, svals, _ = per_core[c]
        slabt = featsT_pad[:, gvals]                 # -1 -> zero column
        in_maps.append(dict(slabt=np.ascontiguousarray(slabt), wt=wt,
                            sidx=_pack_sidx(svals), gb=gb))
    return nc, in_maps, plan


def kernel(feats, W, gamma, beta, in_map, out_map, n_out):
    from concourse.bass_utils import run_bass_kernel_spmd

    n_cores = 8
    nc, in_maps, plan = _prepare(
        feats, W, gamma, beta, in_map, out_map, n_out, n_cores)
    res = run_bass_kernel_spmd(nc, in_maps, list(range(n_cores)))
    rows = plan["rows_per_core"]
    out = np.concatenate(
        [res.results[c]["y"][:rows] for c in range(n_cores)], axis=0)
    return out.astype(np.float32)


# revision 23
# speedup vs baseline: 3.1741x; 1.0846x over previous
"""Trainium2 Bass kernel for nn_BasicDeconvolutionBlock.

Reference computation:
    gathered = feats[in_map]                         # [K, M, Cin]
    contrib  = einsum('kmc,kcd->kmd', gathered, W)   # [K, M, Cout]
    out      = zeros([n_out, Cout]).at[out_map].add(contrib)
    y        = relu(batchnorm(out))                  # batch stats over n_out rows

Strategy (8 NeuronCores, SPMD):
  - Host routes each (k, m) pair to the core owning its output row
    (row blocks of n_out/8) and lowers the gather to im2col: a per-core
    channel-major slab slabT[64, slots] (bf16) holding feats rows in
    k-major slot order, streamed to SBUF with large contiguous DMAs.
  - GEMM: per 128-slot tile (single k per tile), matmul(lhsT=slab tile
    [64ch,128slots], rhs=W[k][64ch,64]) -> PSUM [128slots,64] f32;
    PSUM->SBUF copies alternate between DVE and Activation engines.
  - Scatter: gpsimd dma_scatter_add (CCE-add, int16 idx) accumulates
    contributions into one of two HBM banks, alternating PER SEGMENT so
    adjacent calls have no WAW hazard and fully overlap (desc-gen of
    call i+1 runs during the DMA of call i).  Duplicate output rows
    inside one call race in hardware, so the host spaces a row's
    occurrences ~cnt/h apart in the slot order and swap-fixes the few
    residual in-segment duplicates.  Calls are capped at 896 indices
    (SWDGE Q7 descriptor-ring limit; larger calls wedge the device).
  - BN: fold banks, ones-matmul row sums + sum of squares, [2,64]
    AllReduce across the 8 cores, normalize + ReLU on chip, output
    shard [rows,64] fp32.
"""

import os
import sys

import numpy as np

sys.path.insert(0, "/opt/trn_rl_repo")

import ml_dtypes  # noqa: E402

from concourse import bacc, bass, mybir  # noqa: E402
import concourse.tile as tile  # noqa: E402

BN_EPS = 1e-5
SEG_TILES = int(os.environ.get("DECONV_SEG_TILES", "7"))
SUPER_SEGS = int(os.environ.get("DECONV_SUPER_SEGS", "4"))
NBANKS = int(os.environ.get("DECONV_NBANKS", "3"))
# SWDGE descriptor-ring carveout; ring must hold the in-flight scatter
# calls (NBANKS x SEG_TILES x 128 descriptors)
DMA_SCRATCH = int(os.environ.get("DECONV_SCRATCH", "16384"))
F32 = mybir.dt.float32
BF16 = mybir.dt.bfloat16
I16 = mybir.dt.int16


def _roundup(x, m):
    return (x + m - 1) // m * m


def _order_group(rows):
    """Slot order for one (core, k) group: spread a row's occurrences
    ~cnt/h apart so same-call duplicates are rare. Returns a permutation
    of range(len(rows))."""
    n = len(rows)
    if n == 0:
        return np.empty(0, dtype=np.int64)
    order = np.argsort(rows, kind="stable")
    sr = rows[order]
    first = np.ones(n, dtype=bool)
    first[1:] = sr[1:] != sr[:-1]
    grp = np.cumsum(first) - 1                    # rank of unique row
    grp_start = np.maximum.accumulate(np.where(first, np.arange(n), 0))
    occ = np.arange(n) - grp_start                # occurrence index j
    # occurrence count h per element
    cnt_per_grp = np.bincount(grp)
    h = cnt_per_grp[grp]
    nuniq = cnt_per_grp.size
    key = (occ + grp / max(nuniq, 1)) / h
    final = np.argsort(key, kind="stable")
    return order[final]


def _fix_conflicts(svals, gvals, seg_bounds, group_bounds, dump_row):
    """Ensure no duplicate (non-dump) rows within any segment by swapping
    slots within their k-group. svals/gvals modified in place."""
    nslots = len(svals)
    nseg = len(seg_bounds) - 1
    seg_of = np.zeros(nslots, dtype=np.int64)
    for s in range(nseg):
        seg_of[seg_bounds[s]:seg_bounds[s + 1]] = s
    grp_of = np.zeros(nslots, dtype=np.int64)
    for g in range(len(group_bounds) - 1):
        grp_of[group_bounds[g]:group_bounds[g + 1]] = g

    # per-seg row sets
    seg_sets = [set() for _ in range(nseg)]
    conflicts = []
    is_conflict = np.zeros(nslots, dtype=bool)
    for i in range(nslots):
        r = svals[i]
        if r == dump_row:
            continue
        ss = seg_sets[seg_of[i]]
        if r in ss:
            conflicts.append(i)
            is_conflict[i] = True
        else:
            ss.add(r)
    rng = np.random.default_rng(0)
    for i in conflicts:
        r = int(svals[i])
        g = grp_of[i]
        lo, hi = group_bounds[g], group_bounds[g + 1]
        placed = False
        cands = list(rng.integers(lo, hi, size=200)) + list(range(lo, hi))
        for j in cands:
            j = int(j)
            sj = seg_of[j]
            if sj == seg_of[i] or is_conflict[j]:
                continue
            rj = int(svals[j])
            # after swap: r goes to seg sj, rj comes to seg of i
            if r in seg_sets[sj]:
                continue
            if rj != dump_row:
                if rj == r or rj in seg_sets[seg_of[i]]:
                    continue
            # apply swap
            si = seg_of[i]
            if rj != dump_row:
                seg_sets[sj].discard(rj)
                seg_sets[si].add(rj)
            seg_sets[sj].add(r)
            svals[i], svals[j] = svals[j], svals[i]
            gvals[i], gvals[j] = gvals[j], gvals[i]
            is_conflict[i] = False
            placed = True
            break
        if not placed:
            if os.environ.get("DECONV_ALLOW_SPILL", "0") == "1":
                svals[i] = dump_row  # drops the contribution (test only)
                continue
            raise RuntimeError("conflict fix failed")
    return len(conflicts)


def _route(in_map, out_map, n_out, n_cores):
    """Host-side routing. Returns plan + per-core slot arrays
    (gvals: feats row per slot or -1; svals: local out row per slot)."""
    K, M = in_map.shape
    rows_per_core = n_out // n_cores
    assert rows_per_core * n_cores == n_out
    acc_rows = _roundup(rows_per_core, 128)
    dump_row = acc_rows
    acc_total = acc_rows + 128

    in_flat = in_map.ravel().astype(np.int64)
    out_flat = out_map.ravel().astype(np.int64)
    k_idx = np.repeat(np.arange(K, dtype=np.int64), M)
    core = out_flat // rows_per_core
    row_local = (out_flat - core * rows_per_core).astype(np.int64)

    # per (core, k) counts -> shared caps
    counts = np.zeros((n_cores, K), dtype=np.int64)
    np.add.at(counts, (core, k_idx), 1)
    caps = _roundup(counts.max(axis=0), 128)  # [K]
    group_bounds = np.concatenate([[0], np.cumsum(caps)])
    total_slots = int(group_bounds[-1])

    seg_slots = SEG_TILES * 128
    seg_bounds = list(range(0, total_slots, seg_slots)) + [total_slots]
    if seg_bounds[-1] == seg_bounds[-2]:
        seg_bounds.pop()

    per_core = []
    for c in range(n_cores):
        gvals = np.full(total_slots, -1, dtype=np.int64)
        svals = np.full(total_slots, dump_row, dtype=np.int64)
        sel_c = core == c
        for k in range(K):
            sel = np.nonzero(sel_c & (k_idx == k))[0]
            rows_k = row_local[sel]
            perm = _order_group(rows_k)
            g0 = group_bounds[k]
            n = len(sel)
            gvals[g0:g0 + n] = in_flat[sel][perm]
            svals[g0:g0 + n] = rows_k[perm]
        nfix = _fix_conflicts(svals, gvals, seg_bounds, group_bounds,
                              dump_row)
        per_core.append((gvals, svals, nfix))

    # tile -> k map
    ntiles = total_slots // 128
    tile_k = np.zeros(ntiles, dtype=np.int64)
    for k in range(K):
        tile_k[group_bounds[k] // 128:group_bounds[k + 1] // 128] = k

    plan = dict(
        K=K, rows_per_core=rows_per_core, acc_rows=acc_rows,
        acc_total=acc_total, dump_row=dump_row,
        total_slots=total_slots, ntiles=ntiles, tile_k=tile_k,
        seg_bounds=seg_bounds, seg_slots=seg_slots,
    )
    return plan, per_core


def _build(plan, n_out, n_cores):
    """Trace the Bass program. Returns nc."""
    nc = bacc.Bacc("TRN2", target_bir_lowering=False, debug=False,
                   dynamic_dma_scratch_size=DMA_SCRATCH)

    K = plan["K"]
    acc_rows, acc_total = plan["acc_rows"], plan["acc_total"]
    total_slots = plan["total_slots"]
    tile_k = plan["tile_k"]
    seg_bounds = plan["seg_bounds"]
    nseg = len(seg_bounds) - 1
    Cout = 64

    slabt = nc.dram_tensor("slabt", [64, total_slots], BF16,
                           kind="ExternalInput")
    wt = nc.dram_tensor("wt", [64, K * Cout], BF16, kind="ExternalInput")
    sidx = nc.dram_tensor("sidx", [128, total_slots // 16], I16,
                          kind="ExternalInput")
    gb = nc.dram_tensor("gb", [2, Cout], F32, kind="ExternalInput")
    # bf16 accumulator banks, rows padded to 128 cols so the scatter's
    # 256B row stride holds (elem_step=128, payload=64 cols)
    accs = [nc.dram_tensor(f"acc{b}", [acc_total, 128], BF16)
            for b in range(NBANKS)]
    cc_in = nc.dram_tensor("cc_in", [2, Cout], F32)
    cc_out = nc.dram_tensor("cc_out", [2, Cout], F32, addr_space="Shared")
    y = nc.dram_tensor("y", [acc_rows, Cout], F32, kind="ExternalOutput")

    Tb = acc_rows // 128  # BN column tiles

    # super-segment layout: SUPER_SEGS segments per slab load
    supers = []
    s = 0
    while s < nseg:
        e = min(s + SUPER_SEGS, nseg)
        supers.append((s, e))
        s = e

    with tile.TileContext(nc) as tc:
        with (
            tc.tile_pool(name="const", bufs=1) as cpool,
            tc.tile_pool(name="slab", bufs=3) as slabpool,
            tc.tile_pool(name="oslab", bufs=8) as opool,
            tc.tile_pool(name="sixp", bufs=3) as sixpool,
            tc.tile_pool(name="psum", bufs=8, space="PSUM") as pspool,
        ):
            w_sb = cpool.tile([64, K * Cout], BF16, tag="w")
            nc.sync.dma_start(out=w_sb[:, :], in_=wt[:, :])
            zed = cpool.tile([128, 6400], BF16, tag="zed")
            nc.vector.memset(zed[:, :], 0.0)
            zrows = 128 * 6400 // 128  # 6400 rows per DMA
            # zero acc0 first (gates the first scatter); later banks'
            # zeros are emitted after so they overlap the first supers
            for r0 in range(0, acc_total, zrows):
                rcnt = min(zrows, acc_total - r0)
                nc.sync.dma_start(
                    out=accs[0][r0:r0 + rcnt, :],
                    in_=zed[:, :rcnt],
                )

            first = True
            for (s0seg, s1seg) in supers:
                a = seg_bounds[s0seg]
                b = seg_bounds[s1seg]
                ns_sup = b - a
                g = slabpool.tile([64, SUPER_SEGS * plan["seg_slots"]],
                                  BF16, tag="g")
                nc.sync.dma_start(out=g[:, :ns_sup], in_=slabt[:, a:b])
                si_t = sixpool.tile(
                    [128, SUPER_SEGS * plan["seg_slots"] // 16], I16,
                    tag="si")
                nc.sync.dma_start(
                    out=si_t[:, :ns_sup // 16],
                    in_=sidx[:, a // 16:b // 16],
                )
                if first:
                    # overlap remaining banks' zero-init with super 0
                    for bank in accs[1:]:
                        for r0 in range(0, acc_total, zrows):
                            rcnt = min(zrows, acc_total - r0)
                            nc.sync.dma_start(
                                out=bank[r0:r0 + rcnt, :],
                                in_=zed[:, :rcnt],
                            )
                    first = False
                for seg in range(s0seg, s1seg):
                    sa = seg_bounds[seg]
                    sb = seg_bounds[seg + 1]
                    ns = sb - sa
                    ntile = ns // 128
                    oslab = opool.tile([128, SEG_TILES, Cout], BF16,
                                       tag="oslab")
                    for t in range(ntile):
                        col = (sa - a) + t * 128
                        k = int(tile_k[sa // 128 + t])
                        ps = pspool.tile([128, Cout], F32, tag="ps")
                        nc.tensor.matmul(
                            out=ps[:, :],
                            lhsT=g[:, col:col + 128],
                            rhs=w_sb[:, k * Cout:(k + 1) * Cout],
                            start=True, stop=True,
                        )
                        if t % 2 == 0:
                            nc.vector.tensor_copy(
                                out=oslab[:, t, :], in_=ps[:, :])
                        else:
                            nc.scalar.activation(
                                out=oslab[:, t, :], in_=ps[:, :],
                                func=mybir.ActivationFunctionType.Copy)
                    nc.gpsimd.dma_scatter_add(
                        out_ap=accs[seg % NBANKS][:, 0:Cout],
                        in_ap=oslab[:, :ntile, :],
                        idxs_ap=si_t[:, (sa - a) // 16:(sb - a) // 16],
                        num_idxs=ns,
                        num_idxs_reg=ns,
                        elem_size=Cout,
                        elem_step=128,
                    )

        # ---- BN phase ----
        with (
            tc.tile_pool(name="bn", bufs=1) as bnpool,
            tc.tile_pool(name="bns", bufs=4) as bnspool,
            tc.tile_pool(name="bnp", bufs=2, space="PSUM") as bnps,
        ):
            out_sb = bnpool.tile([128, Tb, 64], F32, tag="outsb")
            CH = 49  # fold chunk (tiles of 128 rows)
            with tc.tile_pool(name="bnc", bufs=2 * NBANKS) as bncpool:
                for c0 in range(0, Tb, CH):
                    cc = min(CH, Tb - c0)
                    chunk_sbs = []
                    for b in range(NBANKS):
                        bsb = bncpool.tile([128, CH, 64], BF16, tag="bchunk")
                        nc.sync.dma_start(
                            out=bsb[:, :cc, :],
                            in_=accs[b][c0 * 128:(c0 + cc) * 128, 0:64])
                        chunk_sbs.append(bsb)
                    nc.vector.tensor_tensor(
                        out=out_sb[:, c0:c0 + cc, :],
                        in0=chunk_sbs[0][:, :cc, :],
                        in1=chunk_sbs[1][:, :cc, :], op=mybir.AluOpType.add)
                    for b in range(2, NBANKS):
                        nc.vector.tensor_tensor(
                            out=out_sb[:, c0:c0 + cc, :],
                            in0=out_sb[:, c0:c0 + cc, :],
                            in1=chunk_sbs[b][:, :cc, :],
                            op=mybir.AluOpType.add)
            ones = bnpool.tile([128, 1], F32, tag="ones")
            nc.vector.memset(ones[:, :], 1.0)
            sum_ps = bnps.tile([1, 64], F32, tag="sum")
            sq_ps = bnps.tile([1, 64], F32, tag="sq")
            for t in range(Tb):
                nc.tensor.matmul(
                    out=sum_ps[:, :], lhsT=ones[:, :], rhs=out_sb[:, t, :],
                    start=(t == 0), stop=(t == Tb - 1),
                )
            sqt = bnspool.tile([128, 64], F32, tag="sqt")
            for t in range(Tb):
                nc.vector.tensor_tensor(
                    out=sqt[:, :], in0=out_sb[:, t, :], in1=out_sb[:, t, :],
                    op=mybir.AluOpType.mult)
                nc.tensor.matmul(
                    out=sq_ps[:, :], lhsT=ones[:, :], rhs=sqt[:, :],
                    start=(t == 0), stop=(t == Tb - 1),
                )
            st0 = bnspool.tile([1, 64], F32, tag="st0")
            st1 = bnspool.tile([1, 64], F32, tag="st1")
            nc.vector.tensor_copy(out=st0[:, :], in_=sum_ps[:, :])
            nc.vector.tensor_copy(out=st1[:, :], in_=sq_ps[:, :])
            nc.sync.dma_start(out=cc_in[0:1, :], in_=st0[:, :])
            nc.sync.dma_start(out=cc_in[1:2, :], in_=st1[:, :])
            nc.gpsimd.collective_compute(
                "AllReduce",
                mybir.AluOpType.add,
                ins=[cc_in[:, :]],
                outs=[cc_out[:, :]],
                replica_groups=[list(range(n_cores))],
            )
            gs0 = bnspool.tile([1, 64], F32, tag="gs0")
            gs1 = bnspool.tile([1, 64], F32, tag="gs1")
            nc.sync.dma_start(out=gs0[:, :], in_=cc_out[0:1, :])
            nc.sync.dma_start(out=gs1[:, :], in_=cc_out[1:2, :])
            gam_t = bnspool.tile([1, 64], F32, tag="gam")
            bet_t = bnspool.tile([1, 64], F32, tag="bet")
            nc.sync.dma_start(out=gam_t[:, :], in_=gb[0:1, :])
            nc.sync.dma_start(out=bet_t[:, :], in_=gb[1:2, :])

            inv_n = 1.0 / float(n_out)
            mean_t = bnspool.tile([1, 64], F32, tag="mean")
            ex2_t = bnspool.tile([1, 64], F32, tag="ex2")
            var_t = bnspool.tile([1, 64], F32, tag="var")
            sd_t = bnspool.tile([1, 64], F32, tag="sd")
            rs_t = bnspool.tile([1, 64], F32, tag="rs")
            a_t = bnspool.tile([1, 64], F32, tag="a")
            b_t = bnspool.tile([1, 64], F32, tag="b")
            nc.vector.tensor_scalar_mul(mean_t[:, :], gs0[:, :], inv_n)
            nc.vector.tensor_scalar_mul(ex2_t[:, :], gs1[:, :], inv_n)
            nc.vector.tensor_tensor(
                out=var_t[:, :], in0=mean_t[:, :], in1=mean_t[:, :],
                op=mybir.AluOpType.mult)
            nc.vector.tensor_tensor(
                out=var_t[:, :], in0=ex2_t[:, :], in1=var_t[:, :],
                op=mybir.AluOpType.subtract)
            nc.vector.tensor_scalar_add(var_t[:, :], var_t[:, :], BN_EPS)
            nc.scalar.activation(
                out=sd_t[:, :], in_=var_t[:, :],
                func=mybir.ActivationFunctionType.Sqrt)
            nc.vector.reciprocal(out=rs_t[:, :], in_=sd_t[:, :])
            nc.vector.tensor_tensor(
                out=a_t[:, :], in0=gam_t[:, :], in1=rs_t[:, :],
                op=mybir.AluOpType.mult)
            nc.vector.tensor_tensor(
                out=b_t[:, :], in0=mean_t[:, :], in1=a_t[:, :],
                op=mybir.AluOpType.mult)
            nc.vector.tensor_tensor(
                out=b_t[:, :], in0=bet_t[:, :], in1=b_t[:, :],
                op=mybir.AluOpType.subtract)
            # broadcast [1,64] -> [128,64] via PE (ones[1,128]^T @ row)
            ones_row = bnspool.tile([1, 128], F32, tag="ones_row")
            nc.vector.memset(ones_row[:, :], 1.0)
            a_full = bnspool.tile([128, 64], F32, tag="afull")
            b_full = bnspool.tile([128, 64], F32, tag="bfull")
            ab_ps = bnps.tile([128, 64], F32, tag="abps")
            nc.tensor.matmul(
                out=ab_ps[:, :], lhsT=ones_row[:, :], rhs=a_t[:, :],
                start=True, stop=True)
            nc.vector.tensor_copy(out=a_full[:, :], in_=ab_ps[:, :])
            nc.tensor.matmul(
                out=ab_ps[:, :], lhsT=ones_row[:, :], rhs=b_t[:, :],
                start=True, stop=True)
            nc.vector.tensor_copy(out=b_full[:, :], in_=ab_ps[:, :])
            for t in range(Tb):
                nc.vector.tensor_tensor(
                    out=out_sb[:, t, :], in0=out_sb[:, t, :], in1=a_full[:, :],
                    op=mybir.AluOpType.mult)
                nc.vector.tensor_tensor(
                    out=out_sb[:, t, :], in0=out_sb[:, t, :], in1=b_full[:, :],
                    op=mybir.AluOpType.add)
                nc.scalar.activation(
                    out=out_sb[:, t, :], in_=out_sb[:, t, :],
                    func=mybir.ActivationFunctionType.Relu)
            # write y with the same chunk-local row mapping as the fold
            for c0 in range(0, Tb, CH):
                cc = min(CH, Tb - c0)
                nc.sync.dma_start(
                    out=y[c0 * 128:(c0 + cc) * 128, :],
                    in_=out_sb[:, c0:c0 + cc, :])

    nc.compile()
    return nc


def _pack_sidx(svals):
    """[total_slots] int -> [128, total_slots//16] int16 wrapped/tiled."""
    cols = svals.reshape(-1, 16).T.astype(np.int16)  # [16, n/16]
    return np.tile(cols, (8, 1))


def _prepare(feats, W, gamma, beta, in_map, out_map, n_out, n_cores=8,
             *_ignored):
    """Host prep shared by kernel() and tests. Returns (nc, in_maps, plan)."""
    n_out = int(n_out)
    K, Cin, Cout = W.shape
    assert Cin == 64 and Cout == 64
    in_map = np.asarray(in_map, dtype=np.int64)
    out_map = np.asarray(out_map, dtype=np.int64)
    feats = np.asarray(feats, dtype=np.float32)
    W = np.asarray(W, dtype=np.float32)

    plan, per_core = _route(in_map, out_map, n_out, n_cores)

    featsT = np.ascontiguousarray(
        feats.T.astype(ml_dtypes.bfloat16))          # [64, N_in]
    featsT_pad = np.concatenate(
        [featsT, np.zeros((64, 1), dtype=ml_dtypes.bfloat16)], axis=1)

    wt = np.ascontiguousarray(
        W.transpose(1, 0, 2).reshape(64, K * 64).astype(ml_dtypes.bfloat16))

    gb = np.stack([np.asarray(gamma, np.float32),
                   np.asarray(beta, np.float32)])

    nc = _build(plan, n_out, n_cores)
    in_maps = []
    for c in range(n_cores):
        gvals, svals, _ = per_core[c]
        slabt = featsT_pad[:, gvals]                 # -1 -> zero column
        in_maps.append(dict(slabt=np.ascontiguousarray(slabt), wt=wt,
                            sidx=_pack_sidx(svals), gb=gb))
    return nc, in_maps, plan


def kernel(feats, W, gamma, beta, in_map, out_map, n_out):
    from concourse.bass_utils import run_bass_kernel_spmd

    n_cores = 8
    nc, in_maps, plan = _prepare(
        feats, W, gamma, beta, in_map, out_map, n_out, n_cores)
    res = run_bass_kernel_spmd(nc, in_maps, list(range(n_cores)))
    rows = plan["rows_per_core"]
    out = np.concatenate(
        [res.results[c]["y"][:rows] for c in range(n_cores)], axis=0)
    return out.astype(np.float32)


# revision 27
# speedup vs baseline: 3.7333x; 1.1762x over previous
"""Trainium2 Bass kernel for nn_BasicDeconvolutionBlock.

Reference computation:
    gathered = feats[in_map]                         # [K, M, Cin]
    contrib  = einsum('kmc,kcd->kmd', gathered, W)   # [K, M, Cout]
    out      = zeros([n_out, Cout]).at[out_map].add(contrib)
    y        = relu(batchnorm(out))                  # batch stats over n_out rows

Strategy (8 NeuronCores, SPMD):
  - Host routes each (k, m) pair to the core owning its output row
    (row blocks of n_out/8) and lowers the gather to im2col: a per-core
    channel-major slab slabT[64, slots] (bf16) holding feats rows in
    k-major slot order, streamed to SBUF with large contiguous DMAs.
  - GEMM: per 128-slot tile (single k per tile), matmul(lhsT=slab tile
    [64ch,128slots], rhs=W[k][64ch,64]) -> PSUM [128slots,64] f32;
    PSUM->SBUF copies alternate between DVE and Activation engines.
  - Scatter: gpsimd dma_scatter_add (CCE-add, int16 idx) accumulates
    contributions into one of two HBM banks, alternating PER SEGMENT so
    adjacent calls have no WAW hazard and fully overlap (desc-gen of
    call i+1 runs during the DMA of call i).  Duplicate output rows
    inside one call race in hardware, so the host spaces a row's
    occurrences ~cnt/h apart in the slot order and swap-fixes the few
    residual in-segment duplicates.  Calls are capped at 896 indices
    (SWDGE Q7 descriptor-ring limit; larger calls wedge the device).
  - BN: fold banks, ones-matmul row sums + sum of squares, [2,64]
    AllReduce across the 8 cores, normalize + ReLU on chip, output
    shard [rows,64] fp32.
"""

import os
import sys

import numpy as np

sys.path.insert(0, "/opt/trn_rl_repo")

import ml_dtypes  # noqa: E402

from concourse import bacc, bass, mybir  # noqa: E402
import concourse.tile as tile  # noqa: E402

BN_EPS = 1e-5
SEG_TILES = int(os.environ.get("DECONV_SEG_TILES", "7"))
SUPER_SEGS = int(os.environ.get("DECONV_SUPER_SEGS", "4"))
NBANKS = int(os.environ.get("DECONV_NBANKS", "3"))
# SWDGE descriptor-ring carveout; ring must hold the in-flight scatter
# calls (NBANKS x SEG_TILES x 128 descriptors)
DMA_SCRATCH = int(os.environ.get("DECONV_SCRATCH", "16384"))
F32 = mybir.dt.float32
BF16 = mybir.dt.bfloat16
I16 = mybir.dt.int16


def _roundup(x, m):
    return (x + m - 1) // m * m


def _order_group(rows):
    """Slot order for one (core, k) group: spread a row's occurrences
    ~cnt/h apart so same-call duplicates are rare. Returns a permutation
    of range(len(rows))."""
    n = len(rows)
    if n == 0:
        return np.empty(0, dtype=np.int64)
    order = np.argsort(rows, kind="stable")
    sr = rows[order]
    first = np.ones(n, dtype=bool)
    first[1:] = sr[1:] != sr[:-1]
    grp = np.cumsum(first) - 1                    # rank of unique row
    grp_start = np.maximum.accumulate(np.where(first, np.arange(n), 0))
    occ = np.arange(n) - grp_start                # occurrence index j
    # occurrence count h per element
    cnt_per_grp = np.bincount(grp)
    h = cnt_per_grp[grp]
    nuniq = cnt_per_grp.size
    key = (occ + grp / max(nuniq, 1)) / h
    final = np.argsort(key, kind="stable")
    return order[final]


def _fix_conflicts(svals, gvals, seg_bounds, group_bounds, dump_row):
    """Ensure no duplicate (non-dump) rows within any segment by swapping
    slots within their k-group. svals/gvals modified in place."""
    nslots = len(svals)
    nseg = len(seg_bounds) - 1
    seg_of = np.zeros(nslots, dtype=np.int64)
    for s in range(nseg):
        seg_of[seg_bounds[s]:seg_bounds[s + 1]] = s
    grp_of = np.zeros(nslots, dtype=np.int64)
    for g in range(len(group_bounds) - 1):
        grp_of[group_bounds[g]:group_bounds[g + 1]] = g

    # per-seg row sets
    seg_sets = [set() for _ in range(nseg)]
    conflicts = []
    is_conflict = np.zeros(nslots, dtype=bool)
    for i in range(nslots):
        r = svals[i]
        if r == dump_row:
            continue
        ss = seg_sets[seg_of[i]]
        if r in ss:
            conflicts.append(i)
            is_conflict[i] = True
        else:
            ss.add(r)
    rng = np.random.default_rng(0)
    for i in conflicts:
        r = int(svals[i])
        g = grp_of[i]
        lo, hi = group_bounds[g], group_bounds[g + 1]
        placed = False
        cands = list(rng.integers(lo, hi, size=200)) + list(range(lo, hi))
        for j in cands:
            j = int(j)
            sj = seg_of[j]
            if sj == seg_of[i] or is_conflict[j]:
                continue
            rj = int(svals[j])
            # after swap: r goes to seg sj, rj comes to seg of i
            if r in seg_sets[sj]:
                continue
            if rj != dump_row:
                if rj == r or rj in seg_sets[seg_of[i]]:
                    continue
            # apply swap
            si = seg_of[i]
            if rj != dump_row:
                seg_sets[sj].discard(rj)
                seg_sets[si].add(rj)
            seg_sets[sj].add(r)
            svals[i], svals[j] = svals[j], svals[i]
            gvals[i], gvals[j] = gvals[j], gvals[i]
            is_conflict[i] = False
            placed = True
            break
        if not placed:
            if os.environ.get("DECONV_ALLOW_SPILL", "0") == "1":
                svals[i] = dump_row  # drops the contribution (test only)
                continue
            raise RuntimeError("conflict fix failed")
    return len(conflicts)


def _route(in_map, out_map, n_out, n_cores):
    """Host-side routing. Returns plan + per-core slot arrays
    (gvals: feats row per slot or -1; svals: local out row per slot)."""
    K, M = in_map.shape
    rows_per_core = n_out // n_cores
    assert rows_per_core * n_cores == n_out
    acc_rows = _roundup(rows_per_core, 128)
    dump_row = acc_rows
    acc_total = acc_rows + 128

    in_flat = in_map.ravel().astype(np.int64)
    out_flat = out_map.ravel().astype(np.int64)
    k_idx = np.repeat(np.arange(K, dtype=np.int64), M)
    core = out_flat // rows_per_core
    row_local = (out_flat - core * rows_per_core).astype(np.int64)

    # per (core, k) counts -> shared caps
    counts = np.zeros((n_cores, K), dtype=np.int64)
    np.add.at(counts, (core, k_idx), 1)
    caps = _roundup(counts.max(axis=0), 128)  # [K]
    group_bounds = np.concatenate([[0], np.cumsum(caps)])
    total_slots = int(group_bounds[-1])

    seg_slots = SEG_TILES * 128
    seg_bounds = list(range(0, total_slots, seg_slots)) + [total_slots]
    if seg_bounds[-1] == seg_bounds[-2]:
        seg_bounds.pop()

    per_core = []
    for c in range(n_cores):
        gvals = np.full(total_slots, -1, dtype=np.int64)
        svals = np.full(total_slots, dump_row, dtype=np.int64)
        sel_c = core == c
        for k in range(K):
            sel = np.nonzero(sel_c & (k_idx == k))[0]
            rows_k = row_local[sel]
            perm = _order_group(rows_k)
            g0 = group_bounds[k]
            n = len(sel)
            gvals[g0:g0 + n] = in_flat[sel][perm]
            svals[g0:g0 + n] = rows_k[perm]
        nfix = _fix_conflicts(svals, gvals, seg_bounds, group_bounds,
                              dump_row)
        per_core.append((gvals, svals, nfix))

    # tile -> k map
    ntiles = total_slots // 128
    tile_k = np.zeros(ntiles, dtype=np.int64)
    for k in range(K):
        tile_k[group_bounds[k] // 128:group_bounds[k + 1] // 128] = k

    plan = dict(
        K=K, rows_per_core=rows_per_core, acc_rows=acc_rows,
        acc_total=acc_total, dump_row=dump_row,
        total_slots=total_slots, ntiles=ntiles, tile_k=tile_k,
        seg_bounds=seg_bounds, seg_slots=seg_slots,
    )
    return plan, per_core


def _build(plan, n_out, n_cores):
    """Trace the Bass program. Returns nc."""
    nc = bacc.Bacc("TRN2", target_bir_lowering=False, debug=False,
                   dynamic_dma_scratch_size=DMA_SCRATCH)

    K = plan["K"]
    acc_rows, acc_total = plan["acc_rows"], plan["acc_total"]
    total_slots = plan["total_slots"]
    tile_k = plan["tile_k"]
    seg_bounds = plan["seg_bounds"]
    nseg = len(seg_bounds) - 1
    Cout = 64

    slabt = nc.dram_tensor("slabt", [64, total_slots], BF16,
                           kind="ExternalInput")
    wt = nc.dram_tensor("wt", [64, K * Cout], BF16, kind="ExternalInput")
    sidx = nc.dram_tensor("sidx", [128, total_slots // 16], I16,
                          kind="ExternalInput")
    gb = nc.dram_tensor("gb", [2, Cout], F32, kind="ExternalInput")
    # bf16 accumulator banks, rows padded to 128 cols so the scatter's
    # 256B row stride holds (elem_step=128, payload=64 cols)
    accs = [nc.dram_tensor(f"acc{b}", [acc_total, 128], BF16)
            for b in range(NBANKS)]
    cc_in = nc.dram_tensor("cc_in", [2, Cout], F32)
    cc_out = nc.dram_tensor("cc_out", [2, Cout], F32, addr_space="Shared")
    y = nc.dram_tensor("y", [acc_rows, Cout], F32, kind="ExternalOutput")

    Tb = acc_rows // 128  # BN column tiles

    # super-segment layout: SUPER_SEGS segments per slab load
    supers = []
    s = 0
    while s < nseg:
        e = min(s + SUPER_SEGS, nseg)
        supers.append((s, e))
        s = e

    with tile.TileContext(nc) as tc:
        with (
            tc.tile_pool(name="const", bufs=1) as cpool,
            tc.tile_pool(name="slab", bufs=3) as slabpool,
            tc.tile_pool(name="oslab", bufs=8) as opool,
            tc.tile_pool(name="sixp", bufs=3) as sixpool,
            tc.tile_pool(name="psum", bufs=8, space="PSUM") as pspool,
        ):
            w_sb = cpool.tile([64, K * Cout], BF16, tag="w")
            nc.sync.dma_start(out=w_sb[:, :], in_=wt[:, :])
            zed = cpool.tile([128, 3200], BF16, tag="zed")
            nc.vector.memset(zed[:, :], 0.0)
            zrows = 6400  # rows per DMA (only the used 64 cols are zeroed)
            # zero acc0 first (gates the first scatter); later banks'
            # zeros are emitted after so they overlap the first supers
            for r0 in range(0, acc_total, zrows):
                rcnt = min(zrows, acc_total - r0)
                nc.sync.dma_start(
                    out=accs[0][r0:r0 + rcnt, 0:64],
                    in_=zed[:, :rcnt // 2],
                )

            first = True
            for (s0seg, s1seg) in supers:
                a = seg_bounds[s0seg]
                b = seg_bounds[s1seg]
                ns_sup = b - a
                g = slabpool.tile([64, SUPER_SEGS * plan["seg_slots"]],
                                  BF16, tag="g")
                nc.sync.dma_start(out=g[:, :ns_sup], in_=slabt[:, a:b])
                si_t = sixpool.tile(
                    [128, SUPER_SEGS * plan["seg_slots"] // 16], I16,
                    tag="si")
                nc.sync.dma_start(
                    out=si_t[:, :ns_sup // 16],
                    in_=sidx[:, a // 16:b // 16],
                )
                if first:
                    # overlap remaining banks' zero-init with super 0
                    for bank in accs[1:]:
                        for r0 in range(0, acc_total, zrows):
                            rcnt = min(zrows, acc_total - r0)
                            nc.sync.dma_start(
                                out=bank[r0:r0 + rcnt, 0:64],
                                in_=zed[:, :rcnt // 2],
                            )
                    first = False
                for seg in range(s0seg, s1seg):
                    sa = seg_bounds[seg]
                    sb = seg_bounds[seg + 1]
                    ns = sb - sa
                    ntile = ns // 128
                    oslab = opool.tile([128, SEG_TILES, Cout], BF16,
                                       tag="oslab")
                    for t in range(ntile):
                        col = (sa - a) + t * 128
                        k = int(tile_k[sa // 128 + t])
                        ps = pspool.tile([128, Cout], F32, tag="ps")
                        nc.tensor.matmul(
                            out=ps[:, :],
                            lhsT=g[:, col:col + 128],
                            rhs=w_sb[:, k * Cout:(k + 1) * Cout],
                            start=True, stop=True,
                        )
                        if t % 2 == 0:
                            nc.vector.tensor_copy(
                                out=oslab[:, t, :], in_=ps[:, :])
                        else:
                            nc.scalar.activation(
                                out=oslab[:, t, :], in_=ps[:, :],
                                func=mybir.ActivationFunctionType.Copy)
                    nc.gpsimd.dma_scatter_add(
                        out_ap=accs[seg % NBANKS][:, 0:Cout],
                        in_ap=oslab[:, :ntile, :],
                        idxs_ap=si_t[:, (sa - a) // 16:(sb - a) // 16],
                        num_idxs=ns,
                        num_idxs_reg=ns,
                        elem_size=Cout,
                        elem_step=128,
                    )

        # ---- BN phase ----
        with (
            tc.tile_pool(name="bn", bufs=1) as bnpool,
            tc.tile_pool(name="bns", bufs=4) as bnspool,
            tc.tile_pool(name="bnp", bufs=2, space="PSUM") as bnps,
        ):
            out_sb = bnpool.tile([128, Tb, 64], BF16, tag="outsb")
            ones = bnpool.tile([128, 1], BF16, tag="ones")
            nc.vector.memset(ones[:, :], 1.0)
            sum_ps = bnps.tile([1, 64], F32, tag="sum")
            sq_ps = bnps.tile([1, 64], F32, tag="sq")
            CH = 49  # fold chunk (tiles of 128 rows)
            with (
                tc.tile_pool(name="bnc", bufs=2 * NBANKS) as bncpool,
                tc.tile_pool(name="bnsq", bufs=4) as sqpool,
            ):
                for c0 in range(0, Tb, CH):
                    cc = min(CH, Tb - c0)
                    chunk_sbs = []
                    for b in range(NBANKS):
                        bsb = bncpool.tile([128, CH, 64], BF16, tag="bchunk")
                        nc.sync.dma_start(
                            out=bsb[:, :cc, :],
                            in_=accs[b][c0 * 128:(c0 + cc) * 128, 0:64])
                        chunk_sbs.append(bsb)
                    nc.vector.tensor_tensor(
                        out=out_sb[:, c0:c0 + cc, :],
                        in0=chunk_sbs[0][:, :cc, :],
                        in1=chunk_sbs[1][:, :cc, :], op=mybir.AluOpType.add)
                    for b in range(2, NBANKS):
                        nc.vector.tensor_tensor(
                            out=out_sb[:, c0:c0 + cc, :],
                            in0=out_sb[:, c0:c0 + cc, :],
                            in1=chunk_sbs[b][:, :cc, :],
                            op=mybir.AluOpType.add)
                    # interleave the stats reductions with the fold
                    for t in range(c0, c0 + cc):
                        sqt = sqpool.tile([128, 64], BF16, tag="sqt")
                        if t % 2 == 0:
                            nc.vector.tensor_tensor(
                                out=sqt[:, :], in0=out_sb[:, t, :],
                                in1=out_sb[:, t, :], op=mybir.AluOpType.mult)
                        else:
                            nc.scalar.activation(
                                out=sqt[:, :], in_=out_sb[:, t, :],
                                func=mybir.ActivationFunctionType.Square)
                        nc.tensor.matmul(
                            out=sum_ps[:, :], lhsT=ones[:, :],
                            rhs=out_sb[:, t, :],
                            start=(t == 0), stop=(t == Tb - 1),
                        )
                        nc.tensor.matmul(
                            out=sq_ps[:, :], lhsT=ones[:, :], rhs=sqt[:, :],
                            start=(t == 0), stop=(t == Tb - 1),
                        )
            st0 = bnspool.tile([1, 64], F32, tag="st0")
            st1 = bnspool.tile([1, 64], F32, tag="st1")
            nc.vector.tensor_copy(out=st0[:, :], in_=sum_ps[:, :])
            nc.vector.tensor_copy(out=st1[:, :], in_=sq_ps[:, :])
            nc.sync.dma_start(out=cc_in[0:1, :], in_=st0[:, :])
            nc.sync.dma_start(out=cc_in[1:2, :], in_=st1[:, :])
            nc.gpsimd.collective_compute(
                "AllReduce",
                mybir.AluOpType.add,
                ins=[cc_in[:, :]],
                outs=[cc_out[:, :]],
                replica_groups=[list(range(n_cores))],
            )
            gs0 = bnspool.tile([1, 64], F32, tag="gs0")
            gs1 = bnspool.tile([1, 64], F32, tag="gs1")
            nc.sync.dma_start(out=gs0[:, :], in_=cc_out[0:1, :])
            nc.sync.dma_start(out=gs1[:, :], in_=cc_out[1:2, :])
            gam_t = bnspool.tile([1, 64], F32, tag="gam")
            bet_t = bnspool.tile([1, 64], F32, tag="bet")
            nc.sync.dma_start(out=gam_t[:, :], in_=gb[0:1, :])
            nc.sync.dma_start(out=bet_t[:, :], in_=gb[1:2, :])

            inv_n = 1.0 / float(n_out)
            mean_t = bnspool.tile([1, 64], F32, tag="mean")
            ex2_t = bnspool.tile([1, 64], F32, tag="ex2")
            var_t = bnspool.tile([1, 64], F32, tag="var")
            sd_t = bnspool.tile([1, 64], F32, tag="sd")
            rs_t = bnspool.tile([1, 64], F32, tag="rs")
            a_t = bnspool.tile([1, 64], F32, tag="a")
            b_t = bnspool.tile([1, 64], F32, tag="b")
            nc.vector.tensor_scalar_mul(mean_t[:, :], gs0[:, :], inv_n)
            nc.vector.tensor_scalar_mul(ex2_t[:, :], gs1[:, :], inv_n)
            nc.vector.tensor_tensor(
                out=var_t[:, :], in0=mean_t[:, :], in1=mean_t[:, :],
                op=mybir.AluOpType.mult)
            nc.vector.tensor_tensor(
                out=var_t[:, :], in0=ex2_t[:, :], in1=var_t[:, :],
                op=mybir.AluOpType.subtract)
            nc.vector.tensor_scalar_add(var_t[:, :], var_t[:, :], BN_EPS)
            nc.scalar.activation(
                out=sd_t[:, :], in_=var_t[:, :],
                func=mybir.ActivationFunctionType.Sqrt)
            nc.vector.reciprocal(out=rs_t[:, :], in_=sd_t[:, :])
            nc.vector.tensor_tensor(
                out=a_t[:, :], in0=gam_t[:, :], in1=rs_t[:, :],
                op=mybir.AluOpType.mult)
            nc.vector.tensor_tensor(
                out=b_t[:, :], in0=mean_t[:, :], in1=a_t[:, :],
                op=mybir.AluOpType.mult)
            nc.vector.tensor_tensor(
                out=b_t[:, :], in0=bet_t[:, :], in1=b_t[:, :],
                op=mybir.AluOpType.subtract)
            # broadcast [1,64] -> [128,64] via PE (ones[1,128]^T @ row)
            ones_row = bnspool.tile([1, 128], F32, tag="ones_row")
            nc.vector.memset(ones_row[:, :], 1.0)
            a_full = bnspool.tile([128, 64], BF16, tag="afull")
            b_full = bnspool.tile([128, 64], BF16, tag="bfull")
            ab_ps = bnps.tile([128, 64], F32, tag="abps")
            nc.tensor.matmul(
                out=ab_ps[:, :], lhsT=ones_row[:, :], rhs=a_t[:, :],
                start=True, stop=True)
            nc.vector.tensor_copy(out=a_full[:, :], in_=ab_ps[:, :])
            nc.tensor.matmul(
                out=ab_ps[:, :], lhsT=ones_row[:, :], rhs=b_t[:, :],
                start=True, stop=True)
            nc.vector.tensor_copy(out=b_full[:, :], in_=ab_ps[:, :])
            # normalize in bf16, relu converts to f32 staging, write chunks
            with tc.tile_pool(name="bny", bufs=2) as ypool:
                for c0 in range(0, Tb, CH):
                    cc = min(CH, Tb - c0)
                    stage = ypool.tile([128, CH, 64], F32, tag="stage")
                    for t in range(c0, c0 + cc):
                        nc.vector.tensor_tensor(
                            out=out_sb[:, t, :], in0=out_sb[:, t, :],
                            in1=a_full[:, :], op=mybir.AluOpType.mult)
                        nc.vector.tensor_tensor(
                            out=out_sb[:, t, :], in0=out_sb[:, t, :],
                            in1=b_full[:, :], op=mybir.AluOpType.add)
                        nc.scalar.activation(
                            out=stage[:, t - c0, :], in_=out_sb[:, t, :],
                            func=mybir.ActivationFunctionType.Relu)
                    nc.sync.dma_start(
                        out=y[c0 * 128:(c0 + cc) * 128, :],
                        in_=stage[:, :cc, :])

    nc.compile()
    return nc


def _pack_sidx(svals):
    """[total_slots] int -> [128, total_slots//16] int16 wrapped/tiled."""
    cols = svals.reshape(-1, 16).T.astype(np.int16)  # [16, n/16]
    return np.tile(cols, (8, 1))


def _prepare(feats, W, gamma, beta, in_map, out_map, n_out, n_cores=8,
             *_ignored):
    """Host prep shared by kernel() and tests. Returns (nc, in_maps, plan)."""
    n_out = int(n_out)
    K, Cin, Cout = W.shape
    assert Cin == 64 and Cout == 64
    in_map = np.asarray(in_map, dtype=np.int64)
    out_map = np.asarray(out_map, dtype=np.int64)
    feats = np.asarray(feats, dtype=np.float32)
    W = np.asarray(W, dtype=np.float32)

    plan, per_core = _route(in_map, out_map, n_out, n_cores)

    featsT = np.ascontiguousarray(
        feats.T.astype(ml_dtypes.bfloat16))          # [64, N_in]
    featsT_pad = np.concatenate(
        [featsT, np.zeros((64, 1), dtype=ml_dtypes.bfloat16)], axis=1)

    wt = np.ascontiguousarray(
        W.transpose(1, 0, 2).reshape(64, K * 64).astype(ml_dtypes.bfloat16))

    gb = np.stack([np.asarray(gamma, np.float32),
                   np.asarray(beta, np.float32)])

    nc = _build(plan, n_out, n_cores)
    in_maps = []
    for c in range(n_cores):
        gvals, svals, _ = per_core[c]
        slabt = featsT_pad[:, gvals]                 # -1 -> zero column
        in_maps.append(dict(slabt=np.ascontiguousarray(slabt), wt=wt,
                            sidx=_pack_sidx(svals), gb=gb))
    return nc, in_maps, plan


def kernel(feats, W, gamma, beta, in_map, out_map, n_out):
    from concourse.bass_utils import run_bass_kernel_spmd

    n_cores = 8
    nc, in_maps, plan = _prepare(
        feats, W, gamma, beta, in_map, out_map, n_out, n_cores)
    res = run_bass_kernel_spmd(nc, in_maps, list(range(n_cores)))
    rows = plan["rows_per_core"]
    out = np.concatenate(
        [res.results[c]["y"][:rows] for c in range(n_cores)], axis=0)
    return out.astype(np.float32)


# revision 30
# speedup vs baseline: 4.0848x; 1.0942x over previous
"""Trainium2 Bass kernel for nn_BasicDeconvolutionBlock.

Reference computation:
    gathered = feats[in_map]                         # [K, M, Cin]
    contrib  = einsum('kmc,kcd->kmd', gathered, W)   # [K, M, Cout]
    out      = zeros([n_out, Cout]).at[out_map].add(contrib)
    y        = relu(batchnorm(out))                  # batch stats over n_out rows

Strategy (8 NeuronCores, SPMD):
  - Host routes each (k, m) pair to the core owning its output row
    (row blocks of n_out/8) and lowers the gather to im2col: a per-core
    channel-major slab slabT[64, slots] (bf16) holding feats rows in
    k-major slot order, streamed to SBUF with large contiguous DMAs.
  - GEMM: per 128-slot tile (single k per tile), matmul(lhsT=slab tile
    [64ch,128slots], rhs=W[k][64ch,64]) -> PSUM [128slots,64] f32;
    PSUM->SBUF copies alternate between DVE and Activation engines.
  - Scatter: gpsimd dma_scatter_add (CCE-add, int16 idx) accumulates
    contributions into one of two HBM banks, alternating PER SEGMENT so
    adjacent calls have no WAW hazard and fully overlap (desc-gen of
    call i+1 runs during the DMA of call i).  Duplicate output rows
    inside one call race in hardware, so the host spaces a row's
    occurrences ~cnt/h apart in the slot order and swap-fixes the few
    residual in-segment duplicates.  Calls are capped at 896 indices
    (SWDGE Q7 descriptor-ring limit; larger calls wedge the device).
  - BN: fold banks, ones-matmul row sums + sum of squares, [2,64]
    AllReduce across the 8 cores, normalize + ReLU on chip, output
    shard [rows,64] fp32.
"""

import os
import sys

import numpy as np

sys.path.insert(0, "/opt/trn_rl_repo")

import ml_dtypes  # noqa: E402

from concourse import bacc, bass, mybir  # noqa: E402
import concourse.tile as tile  # noqa: E402

BN_EPS = 1e-5
SEG_TILES = int(os.environ.get("DECONV_SEG_TILES", "7"))
SUPER_SEGS = int(os.environ.get("DECONV_SUPER_SEGS", "4"))
NBANKS = int(os.environ.get("DECONV_NBANKS", "3"))
# SWDGE descriptor-ring carveout; ring must hold the in-flight scatter
# calls (NBANKS x SEG_TILES x 128 descriptors)
DMA_SCRATCH = int(os.environ.get("DECONV_SCRATCH", "16384"))
F32 = mybir.dt.float32
BF16 = mybir.dt.bfloat16
I16 = mybir.dt.int16


def _roundup(x, m):
    return (x + m - 1) // m * m


def _order_group(rows):
    """Slot order for one (core, k) group: spread a row's occurrences
    ~cnt/h apart so same-call duplicates are rare. Returns a permutation
    of range(len(rows))."""
    n = len(rows)
    if n == 0:
        return np.empty(0, dtype=np.int64)
    order = np.argsort(rows, kind="stable")
    sr = rows[order]
    first = np.ones(n, dtype=bool)
    first[1:] = sr[1:] != sr[:-1]
    grp = np.cumsum(first) - 1                    # rank of unique row
    grp_start = np.maximum.accumulate(np.where(first, np.arange(n), 0))
    occ = np.arange(n) - grp_start                # occurrence index j
    # occurrence count h per element
    cnt_per_grp = np.bincount(grp)
    h = cnt_per_grp[grp]
    nuniq = cnt_per_grp.size
    key = (occ + grp / max(nuniq, 1)) / h
    final = np.argsort(key, kind="stable")
    return order[final]


def _fix_conflicts(svals, gvals, seg_bounds, group_bounds, dump_row):
    """Ensure no duplicate (non-dump) rows within any segment by swapping
    slots within their k-group. svals/gvals modified in place."""
    nslots = len(svals)
    nseg = len(seg_bounds) - 1
    seg_of = np.zeros(nslots, dtype=np.int64)
    for s in range(nseg):
        seg_of[seg_bounds[s]:seg_bounds[s + 1]] = s
    grp_of = np.zeros(nslots, dtype=np.int64)
    for g in range(len(group_bounds) - 1):
        grp_of[group_bounds[g]:group_bounds[g + 1]] = g

    # per-seg row sets
    seg_sets = [set() for _ in range(nseg)]
    conflicts = []
    failed = set()
    is_conflict = np.zeros(nslots, dtype=bool)
    for i in range(nslots):
        r = svals[i]
        if r == dump_row:
            continue
        ss = seg_sets[seg_of[i]]
        if r in ss:
            conflicts.append(i)
            is_conflict[i] = True
        else:
            ss.add(r)
    rng = np.random.default_rng(0)
    for i in conflicts:
        r = int(svals[i])
        g = grp_of[i]
        lo, hi = group_bounds[g], group_bounds[g + 1]
        placed = False
        cands = list(rng.integers(lo, hi, size=200)) + list(range(lo, hi))
        for j in cands:
            j = int(j)
            sj = seg_of[j]
            if sj == seg_of[i] or is_conflict[j]:
                continue
            rj = int(svals[j])
            # after swap: r goes to seg sj, rj comes to seg of i
            if r in seg_sets[sj]:
                continue
            if rj != dump_row:
                if rj == r or rj in seg_sets[seg_of[i]]:
                    continue
            # apply swap
            si = seg_of[i]
            if rj != dump_row:
                seg_sets[sj].discard(rj)
                seg_sets[si].add(rj)
            seg_sets[sj].add(r)
            svals[i], svals[j] = svals[j], svals[i]
            gvals[i], gvals[j] = gvals[j], gvals[i]
            is_conflict[i] = False
            placed = True
            break
        if not placed:
            failed.add(int(seg_of[i]))
    return len(conflicts), failed


def _route(in_map, out_map, n_out, n_cores):
    """Host-side routing. Returns plan + per-core slot arrays
    (gvals: feats row per slot or -1; svals: local out row per slot)."""
    K, M = in_map.shape
    rows_per_core = n_out // n_cores
    assert rows_per_core * n_cores == n_out
    acc_rows = _roundup(rows_per_core, 128)
    dump_row = acc_rows
    acc_total = acc_rows + 128

    in_flat = in_map.ravel().astype(np.int64)
    out_flat = out_map.ravel().astype(np.int64)
    k_idx = np.repeat(np.arange(K, dtype=np.int64), M)
    core = out_flat // rows_per_core
    row_local = (out_flat - core * rows_per_core).astype(np.int64)

    # per (core, k) counts -> shared caps
    counts = np.zeros((n_cores, K), dtype=np.int64)
    np.add.at(counts, (core, k_idx), 1)
    caps = _roundup(counts.max(axis=0), 128)  # [K]
    group_bounds = np.concatenate([[0], np.cumsum(caps)])
    total_slots = int(group_bounds[-1])

    seg_slots = SEG_TILES * 128
    seg_bounds = list(range(0, total_slots, seg_slots)) + [total_slots]
    if seg_bounds[-1] == seg_bounds[-2]:
        seg_bounds.pop()

    # iteratively split segments whose duplicate conflicts can't be
    # swap-fixed (rare: rows occupying every window their group spans)
    for _ in range(8):
        per_core = []
        all_failed = set()
        for c in range(n_cores):
            gvals = np.full(total_slots, -1, dtype=np.int64)
            svals = np.full(total_slots, dump_row, dtype=np.int64)
            sel_c = core == c
            for k in range(K):
                sel = np.nonzero(sel_c & (k_idx == k))[0]
                rows_k = row_local[sel]
                perm = _order_group(rows_k)
                g0 = group_bounds[k]
                n = len(sel)
                gvals[g0:g0 + n] = in_flat[sel][perm]
                svals[g0:g0 + n] = rows_k[perm]
            nfix, failed = _fix_conflicts(svals, gvals, seg_bounds,
                                          group_bounds, dump_row)
            per_core.append((gvals, svals, nfix))
            all_failed |= failed
        if not all_failed:
            break
        new_bounds = []
        for s in range(len(seg_bounds) - 1):
            a, b = seg_bounds[s], seg_bounds[s + 1]
            new_bounds.append(a)
            if s in all_failed and b - a >= 256:
                new_bounds.append(a + (b - a) // 256 * 128)
        new_bounds.append(seg_bounds[-1])
        seg_bounds = new_bounds
    else:
        raise RuntimeError("segment splitting did not converge")

    # tile -> k map
    ntiles = total_slots // 128
    tile_k = np.zeros(ntiles, dtype=np.int64)
    for k in range(K):
        tile_k[group_bounds[k] // 128:group_bounds[k + 1] // 128] = k

    plan = dict(
        K=K, rows_per_core=rows_per_core, acc_rows=acc_rows,
        acc_total=acc_total, dump_row=dump_row,
        total_slots=total_slots, ntiles=ntiles, tile_k=tile_k,
        seg_bounds=seg_bounds, seg_slots=seg_slots,
    )
    return plan, per_core


def _build(plan, n_out, n_cores):
    """Trace the Bass program. Returns nc."""
    nc = bacc.Bacc("TRN2", target_bir_lowering=False, debug=False,
                   dynamic_dma_scratch_size=DMA_SCRATCH)

    K = plan["K"]
    acc_rows, acc_total = plan["acc_rows"], plan["acc_total"]
    total_slots = plan["total_slots"]
    tile_k = plan["tile_k"]
    seg_bounds = plan["seg_bounds"]
    nseg = len(seg_bounds) - 1
    Cout = 64

    slabt = nc.dram_tensor("slabt", [64, total_slots], BF16,
                           kind="ExternalInput")
    wt = nc.dram_tensor("wt", [64, K * Cout], BF16, kind="ExternalInput")
    sidx = nc.dram_tensor("sidx", [128, total_slots // 16], I16,
                          kind="ExternalInput")
    gb = nc.dram_tensor("gb", [2, Cout], F32, kind="ExternalInput")
    # bf16 accumulator banks, rows padded to 128 cols so the scatter's
    # 256B row stride holds (elem_step=128, payload=64 cols)
    accs = [nc.dram_tensor(f"acc{b}", [acc_total, 128], BF16)
            for b in range(NBANKS)]
    cc_in = nc.dram_tensor("cc_in", [2, Cout], F32)
    cc_out = nc.dram_tensor("cc_out", [2, Cout], F32, addr_space="Shared")
    y = nc.dram_tensor("y", [acc_rows, Cout], F32, kind="ExternalOutput")

    Tb = acc_rows // 128  # BN column tiles

    # super-segment layout: SUPER_SEGS segments per slab load
    supers = []
    s = 0
    while s < nseg:
        e = min(s + SUPER_SEGS, nseg)
        supers.append((s, e))
        s = e

    with tile.TileContext(nc) as tc:
        with (
            tc.tile_pool(name="const", bufs=1) as cpool,
            tc.tile_pool(name="slab", bufs=3) as slabpool,
            tc.tile_pool(name="oslab", bufs=8) as opool,
            tc.tile_pool(name="sixp", bufs=3) as sixpool,
            tc.tile_pool(name="psum", bufs=8, space="PSUM") as pspool,
        ):
            w_sb = cpool.tile([64, K * Cout], BF16, tag="w")
            nc.sync.dma_start(out=w_sb[:, :], in_=wt[:, :])
            zed = cpool.tile([128, 3200], BF16, tag="zed")
            nc.vector.memset(zed[:, :], 0.0)
            zrows = 6400  # rows per DMA (only the used 64 cols are zeroed)
            # zero acc0 first (gates the first scatter); later banks'
            # zeros are emitted after so they overlap the first supers
            for r0 in range(0, acc_total, zrows):
                rcnt = min(zrows, acc_total - r0)
                nc.sync.dma_start(
                    out=accs[0][r0:r0 + rcnt, 0:64],
                    in_=zed[:, :rcnt // 2],
                )

            first = True
            for (s0seg, s1seg) in supers:
                a = seg_bounds[s0seg]
                b = seg_bounds[s1seg]
                ns_sup = b - a
                g = slabpool.tile([64, SUPER_SEGS * plan["seg_slots"]],
                                  BF16, tag="g")
                nc.sync.dma_start(out=g[:, :ns_sup], in_=slabt[:, a:b])
                si_t = sixpool.tile(
                    [128, SUPER_SEGS * plan["seg_slots"] // 16], I16,
                    tag="si")
                nc.sync.dma_start(
                    out=si_t[:, :ns_sup // 16],
                    in_=sidx[:, a // 16:b // 16],
                )
                if first:
                    # overlap remaining banks' zero-init with super 0
                    for bank in accs[1:]:
                        for r0 in range(0, acc_total, zrows):
                            rcnt = min(zrows, acc_total - r0)
                            nc.sync.dma_start(
                                out=bank[r0:r0 + rcnt, 0:64],
                                in_=zed[:, :rcnt // 2],
                            )
                    first = False
                for seg in range(s0seg, s1seg):
                    sa = seg_bounds[seg]
                    sb = seg_bounds[seg + 1]
                    ns = sb - sa
                    ntile = ns // 128
                    oslab = opool.tile([128, SEG_TILES, Cout], BF16,
                                       tag="oslab")
                    for t in range(ntile):
                        col = (sa - a) + t * 128
                        k = int(tile_k[sa // 128 + t])
                        ps = pspool.tile([128, Cout], F32, tag="ps")
                        nc.tensor.matmul(
                            out=ps[:, :],
                            lhsT=g[:, col:col + 128],
                            rhs=w_sb[:, k * Cout:(k + 1) * Cout],
                            start=True, stop=True,
                        )
                        if t % 2 == 0:
                            nc.vector.tensor_copy(
                                out=oslab[:, t, :], in_=ps[:, :])
                        else:
                            nc.scalar.activation(
                                out=oslab[:, t, :], in_=ps[:, :],
                                func=mybir.ActivationFunctionType.Copy)
                    nc.gpsimd.dma_scatter_add(
                        out_ap=accs[seg % NBANKS][:, 0:Cout],
                        in_ap=oslab[:, :ntile, :],
                        idxs_ap=si_t[:, (sa - a) // 16:(sb - a) // 16],
                        num_idxs=ns,
                        num_idxs_reg=ns,
                        elem_size=Cout,
                        elem_step=128,
                    )

        # ---- BN phase ----
        with (
            tc.tile_pool(name="bn", bufs=1) as bnpool,
            tc.tile_pool(name="bns", bufs=4) as bnspool,
            tc.tile_pool(name="bnp", bufs=2, space="PSUM") as bnps,
        ):
            out_sb = bnpool.tile([128, Tb, 64], BF16, tag="outsb")
            ones = bnpool.tile([128, 1], BF16, tag="ones")
            nc.vector.memset(ones[:, :], 1.0)
            sum_ps = bnps.tile([1, 64], F32, tag="sum")
            sq_ps = bnps.tile([1, 64], F32, tag="sq")
            CH = 49  # fold chunk (tiles of 128 rows)
            with (
                tc.tile_pool(name="bnc", bufs=2 * NBANKS) as bncpool,
                tc.tile_pool(name="bnsq", bufs=4) as sqpool,
            ):
                for c0 in range(0, Tb, CH):
                    cc = min(CH, Tb - c0)
                    chunk_sbs = []
                    for b in range(NBANKS):
                        bsb = bncpool.tile([128, CH, 64], BF16, tag="bchunk")
                        nc.sync.dma_start(
                            out=bsb[:, :cc, :],
                            in_=accs[b][c0 * 128:(c0 + cc) * 128, 0:64])
                        chunk_sbs.append(bsb)
                    nc.vector.tensor_tensor(
                        out=out_sb[:, c0:c0 + cc, :],
                        in0=chunk_sbs[0][:, :cc, :],
                        in1=chunk_sbs[1][:, :cc, :], op=mybir.AluOpType.add)
                    for b in range(2, NBANKS):
                        nc.vector.tensor_tensor(
                            out=out_sb[:, c0:c0 + cc, :],
                            in0=out_sb[:, c0:c0 + cc, :],
                            in1=chunk_sbs[b][:, :cc, :],
                            op=mybir.AluOpType.add)
                    # interleave the stats reductions with the fold
                    for t in range(c0, c0 + cc):
                        sqt = sqpool.tile([128, 64], BF16, tag="sqt")
                        if t % 2 == 0:
                            nc.vector.tensor_tensor(
                                out=sqt[:, :], in0=out_sb[:, t, :],
                                in1=out_sb[:, t, :], op=mybir.AluOpType.mult)
                        else:
                            nc.scalar.activation(
                                out=sqt[:, :], in_=out_sb[:, t, :],
                                func=mybir.ActivationFunctionType.Square)
                        nc.tensor.matmul(
                            out=sum_ps[:, :], lhsT=ones[:, :],
                            rhs=out_sb[:, t, :],
                            start=(t == 0), stop=(t == Tb - 1),
                        )
                        nc.tensor.matmul(
                            out=sq_ps[:, :], lhsT=ones[:, :], rhs=sqt[:, :],
                            start=(t == 0), stop=(t == Tb - 1),
                        )
            st0 = bnspool.tile([1, 64], F32, tag="st0")
            st1 = bnspool.tile([1, 64], F32, tag="st1")
            nc.vector.tensor_copy(out=st0[:, :], in_=sum_ps[:, :])
            nc.vector.tensor_copy(out=st1[:, :], in_=sq_ps[:, :])
            nc.sync.dma_start(out=cc_in[0:1, :], in_=st0[:, :])
            nc.sync.dma_start(out=cc_in[1:2, :], in_=st1[:, :])
            nc.gpsimd.collective_compute(
                "AllReduce",
                mybir.AluOpType.add,
                ins=[cc_in[:, :]],
                outs=[cc_out[:, :]],
                replica_groups=[list(range(n_cores))],
            )
            gs0 = bnspool.tile([1, 64], F32, tag="gs0")
            gs1 = bnspool.tile([1, 64], F32, tag="gs1")
            nc.sync.dma_start(out=gs0[:, :], in_=cc_out[0:1, :])
            nc.sync.dma_start(out=gs1[:, :], in_=cc_out[1:2, :])
            gam_t = bnspool.tile([1, 64], F32, tag="gam")
            bet_t = bnspool.tile([1, 64], F32, tag="bet")
            nc.sync.dma_start(out=gam_t[:, :], in_=gb[0:1, :])
            nc.sync.dma_start(out=bet_t[:, :], in_=gb[1:2, :])

            inv_n = 1.0 / float(n_out)
            mean_t = bnspool.tile([1, 64], F32, tag="mean")
            ex2_t = bnspool.tile([1, 64], F32, tag="ex2")
            var_t = bnspool.tile([1, 64], F32, tag="var")
            sd_t = bnspool.tile([1, 64], F32, tag="sd")
            rs_t = bnspool.tile([1, 64], F32, tag="rs")
            a_t = bnspool.tile([1, 64], F32, tag="a")
            b_t = bnspool.tile([1, 64], F32, tag="b")
            nc.vector.tensor_scalar_mul(mean_t[:, :], gs0[:, :], inv_n)
            nc.vector.tensor_scalar_mul(ex2_t[:, :], gs1[:, :], inv_n)
            nc.vector.tensor_tensor(
                out=var_t[:, :], in0=mean_t[:, :], in1=mean_t[:, :],
                op=mybir.AluOpType.mult)
            nc.vector.tensor_tensor(
                out=var_t[:, :], in0=ex2_t[:, :], in1=var_t[:, :],
                op=mybir.AluOpType.subtract)
            nc.vector.tensor_scalar_add(var_t[:, :], var_t[:, :], BN_EPS)
            nc.scalar.activation(
                out=sd_t[:, :], in_=var_t[:, :],
                func=mybir.ActivationFunctionType.Sqrt)
            nc.vector.reciprocal(out=rs_t[:, :], in_=sd_t[:, :])
            nc.vector.tensor_tensor(
                out=a_t[:, :], in0=gam_t[:, :], in1=rs_t[:, :],
                op=mybir.AluOpType.mult)
            nc.vector.tensor_tensor(
                out=b_t[:, :], in0=mean_t[:, :], in1=a_t[:, :],
                op=mybir.AluOpType.mult)
            nc.vector.tensor_tensor(
                out=b_t[:, :], in0=bet_t[:, :], in1=b_t[:, :],
                op=mybir.AluOpType.subtract)
            # broadcast [1,64] -> [128,64] via PE (ones[1,128]^T @ row)
            ones_row = bnspool.tile([1, 128], F32, tag="ones_row")
            nc.vector.memset(ones_row[:, :], 1.0)
            a_full = bnspool.tile([128, 64], BF16, tag="afull")
            b_full = bnspool.tile([128, 64], BF16, tag="bfull")
            ab_ps = bnps.tile([128, 64], F32, tag="abps")
            nc.tensor.matmul(
                out=ab_ps[:, :], lhsT=ones_row[:, :], rhs=a_t[:, :],
                start=True, stop=True)
            nc.vector.tensor_copy(out=a_full[:, :], in_=ab_ps[:, :])
            nc.tensor.matmul(
                out=ab_ps[:, :], lhsT=ones_row[:, :], rhs=b_t[:, :],
                start=True, stop=True)
            nc.vector.tensor_copy(out=b_full[:, :], in_=ab_ps[:, :])
            # normalize in bf16, relu converts to f32 staging, write chunks
            with tc.tile_pool(name="bny", bufs=2) as ypool:
                for c0 in range(0, Tb, CH):
                    cc = min(CH, Tb - c0)
                    stage = ypool.tile([128, CH, 64], F32, tag="stage")
                    for t in range(c0, c0 + cc):
                        nc.vector.tensor_tensor(
                            out=out_sb[:, t, :], in0=out_sb[:, t, :],
                            in1=a_full[:, :], op=mybir.AluOpType.mult)
                        nc.vector.tensor_tensor(
                            out=out_sb[:, t, :], in0=out_sb[:, t, :],
                            in1=b_full[:, :], op=mybir.AluOpType.add)
                        nc.scalar.activation(
                            out=stage[:, t - c0, :], in_=out_sb[:, t, :],
                            func=mybir.ActivationFunctionType.Relu)
                    nc.sync.dma_start(
                        out=y[c0 * 128:(c0 + cc) * 128, :],
                        in_=stage[:, :cc, :])

    nc.compile()
    return nc


def _pack_sidx(svals):
    """[total_slots] int -> [128, total_slots//16] int16 wrapped/tiled."""
    cols = svals.reshape(-1, 16).T.astype(np.int16)  # [16, n/16]
    return np.tile(cols, (8, 1))


def _prepare(feats, W, gamma, beta, in_map, out_map, n_out, n_cores=8,
             *_ignored):
    """Host prep shared by kernel() and tests. Returns (nc, in_maps, plan)."""
    n_out = int(n_out)
    K, Cin, Cout = W.shape
    assert Cin == 64 and Cout == 64
    in_map = np.asarray(in_map, dtype=np.int64)
    out_map = np.asarray(out_map, dtype=np.int64)
    feats = np.asarray(feats, dtype=np.float32)
    W = np.asarray(W, dtype=np.float32)

    plan, per_core = _route(in_map, out_map, n_out, n_cores)

    featsT = np.ascontiguousarray(
        feats.T.astype(ml_dtypes.bfloat16))          # [64, N_in]
    featsT_pad = np.concatenate(
        [featsT, np.zeros((64, 1), dtype=ml_dtypes.bfloat16)], axis=1)

    wt = np.ascontiguousarray(
        W.transpose(1, 0, 2).reshape(64, K * 64).astype(ml_dtypes.bfloat16))

    gb = np.stack([np.asarray(gamma, np.float32),
                   np.asarray(beta, np.float32)])

    nc = _build(plan, n_out, n_cores)
    in_maps = []
    for c in range(n_cores):
        gvals, svals, _ = per_core[c]
        slabt = featsT_pad[:, gvals]                 # -1 -> zero column
        in_maps.append(dict(slabt=np.ascontiguousarray(slabt), wt=wt,
                            sidx=_pack_sidx(svals), gb=gb))
    return nc, in_maps, plan


def kernel(feats, W, gamma, beta, in_map, out_map, n_out):
    from concourse.bass_utils import run_bass_kernel_spmd

    n_cores = 8
    nc, in_maps, plan = _prepare(
        feats, W, gamma, beta, in_map, out_map, n_out, n_cores)
    res = run_bass_kernel_spmd(nc, in_maps, list(range(n_cores)))
    rows = plan["rows_per_core"]
    out = np.concatenate(
        [res.results[c]["y"][:rows] for c in range(n_cores)], axis=0)
    return out.astype(np.float32)
